# revision 16
# baseline (speedup 1.0000x reference)
"""NodeNet GNN message-passing kernel for 8 Trainium2 NeuronCores.

Strategy (per sharding hint): shard nodes across the 8 cores; partition
edges by destination node on the host so the scatter-mean is device-local.

Per core (12,500 real nodes, padded to 12,544 = 196 windows of 64 nodes):
  - Host sorts edges by destination, pre-scales each edge row by
    1/count(dst), pads each 64-node window's edge list to a multiple of
    128, and lays edge features out chunk-transposed ([128, nch*128]) so
    all device DMAs are wide and contiguous.
  - Device builds, per 128-edge chunk, a [128 edge, 64 node] one-hot
    (is_equal against an iota ramp) and uses the TensorEngine to compute
    meanT[d, n] += attr_chunk.T-contract: matmul(lhsT=attr[e,d],
    rhs=onehot[e,n]) accumulated in PSUM.  Everything stays
    feature-major so the 3-layer MLP chains without transposes:
    h1T = relu(W1.T @ [xT; meanT] + b1), etc.
  - Output is written feature-major [128, 12544]; host transposes back.
"""

import numpy as np

import concourse.bass as bass
import concourse.bacc as bacc
import concourse.mybir as mybir
import concourse.tile as tile
from concourse.bass_utils import run_bass_kernel_spmd

P = 128                    # partitions / matmul contraction tile
D = 128                    # node & edge feature dim
HIDDEN = 256
DOUT = 128
N_NODES = 100000
N_CORES = 8
NPC_REAL = 12500           # real nodes per core
W = 64                     # nodes per binning window
WINDOWS = 196              # windows per core (196*64 = 12544)
NPC = WINDOWS * W          # padded nodes per core
GPW = 8                    # windows per MLP group (512 nodes)
GROUP_N = GPW * W

_prog_cache: dict = {}

f32 = mybir.dt.float32
f16 = mybir.dt.float16
f32r = mybir.dt.float32r


def _const_layout(CBmax, NCH):
    """Column offsets in the constants blob:
    W1k0|W1k1|W2k0|W2k1|W3k0|W3k1|b1a|b1b|b2a|b2b|b3|iota_rep|dstrel."""
    CO_B = 0
    CO_DST = CO_B + 5
    CONSTW = CO_DST + NCH
    return CO_B, CO_DST, CONSTW


def _build_program(CB, ablate=()):
    """Build the Bass/Tile program. CB[j] = number of 128-edge chunks for
    window j (identical across cores; per-core data is padded to match).
    ablate: subset of {"mlp", "bin", "oh"} to skip stages (sim studies)."""
    CB = list(CB)
    CBmax = max(CB)
    offs = np.concatenate([[0], np.cumsum(CB)]).astype(int)
    NCH = int(offs[-1])

    CO_B, CO_DST, CONSTW = _const_layout(CBmax, NCH)

    nc = bacc.Bacc(None)
    xT_d = nc.dram_tensor("xT", [P, NPC], f32r, kind="ExternalInput")
    attrT_d = nc.dram_tensor("attrT", [P, NCH * D], f16, kind="ExternalInput")
    consts_d = nc.dram_tensor("consts", [P, CONSTW], f32, kind="ExternalInput")
    iota16_d = nc.dram_tensor("iota16", [P, CBmax * W], f16, kind="ExternalInput")
    wts_d = nc.dram_tensor("wts", [P, 4 * HIDDEN + 2 * DOUT], f32r, kind="ExternalInput")
    outT_d = nc.dram_tensor("outT", [P, NPC], f32, kind="ExternalOutput")

    Relu = mybir.ActivationFunctionType.Relu
    Ident = mybir.ActivationFunctionType.Identity

    with tile.TileContext(nc) as tc:
        with (
            tc.tile_pool(name="const", bufs=1) as cpool,
            tc.tile_pool(name="attr", bufs=3) as apool,
            tc.tile_pool(name="oh", bufs=3) as ohpool,
            tc.tile_pool(name="acts", bufs=2) as actpool,
            tc.tile_pool(name="pbin", bufs=2, space="PSUM") as pbin,
            tc.tile_pool(name="pmlp", bufs=1, space="PSUM") as pmlp,
        ):
            # --- constants: one blob, one DMA, one semaphore ---
            cs = cpool.tile([P, CONSTW], f32, tag="consts")
            nc.sync.dma_start(out=cs[:], in_=consts_d[:, :])
            ws = cpool.tile([P, 4 * HIDDEN + 2 * DOUT], f32r, tag="wts")
            nc.sync.dma_start(out=ws[:], in_=wts_d[:, :])
            w1s_0 = ws[:, 0:HIDDEN]
            w1s_1 = ws[:, HIDDEN : 2 * HIDDEN]
            w2s_0 = ws[:, 2 * HIDDEN : 3 * HIDDEN]
            w2s_1 = ws[:, 3 * HIDDEN : 4 * HIDDEN]
            w3s_0 = ws[:, 4 * HIDDEN : 4 * HIDDEN + DOUT]
            w3s_1 = ws[:, 4 * HIDDEN + DOUT : 4 * HIDDEN + 2 * DOUT]
            b1s_0 = cs[:, CO_B : CO_B + 1]
            b1s_1 = cs[:, CO_B + 1 : CO_B + 2]
            b2s_0 = cs[:, CO_B + 2 : CO_B + 3]
            b2s_1 = cs[:, CO_B + 3 : CO_B + 4]
            b3s = cs[:, CO_B + 4 : CO_B + 5]
            dstrel_s = cs[:, CO_DST : CO_DST + NCH]
            it16 = cpool.tile([P, CBmax * W], f16, tag="iota16")
            nc.sync.dma_start(out=it16[:], in_=iota16_d[:, :])

            for j in range(WINDOWS):
                cb = CB[j]
                off = int(offs[j])
                g, sw = divmod(j, GPW)

                at = apool.tile([P, CBmax * D], f16, tag="attr")
                nc.sync.dma_start(
                    out=at[:, : cb * D], in_=attrT_d[:, off * D : (off + cb) * D]
                )

                oh = ohpool.tile([P, CBmax * W], f16, tag="oh")
                if "oh" not in ablate:
                    nc.vector.tensor_tensor(
                    out=oh[:, : cb * W].rearrange("p (c m) -> p c m", m=W),
                    in0=dstrel_s[:, off : off + cb].to_broadcast([P, cb, W]),
                    in1=it16[:, : cb * W].rearrange("p (c m) -> p c m", m=W),
                        op=mybir.AluOpType.is_equal,
                    )

                pm = pbin.tile([P, W], f32, tag="mean")
                for ch in range(cb if "bin" not in ablate else 0):
                    nc.tensor.matmul(
                        out=pm[:],
                        lhsT=at[:, ch * D : (ch + 1) * D],
                        rhs=oh[:, ch * W : (ch + 1) * W],
                        start=(ch == 0),
                        stop=(ch == cb - 1),
                    )

                if sw == 0:
                    mean_g = actpool.tile([P, GROUP_N], f32r, tag="mean_g")
                if "bin" not in ablate:
                    nc.scalar.copy(out=mean_g[:, sw * W : (sw + 1) * W], in_=pm[:])

                if ("mlp" not in ablate) and (sw == GPW - 1 or j == WINDOWS - 1):
                    # --- MLP over this group of nodes (feature-major) ---
                    NW = (sw + 1) * W
                    n0 = g * GROUP_N
                    xg = actpool.tile([P, GROUP_N], f32r, tag="xg")
                    nc.sync.dma_start(out=xg[:, :NW], in_=xT_d[:, n0 : n0 + NW])

                    ph1a = pmlp.tile([P, GROUP_N], f32, tag="h1a")
                    ph1b = pmlp.tile([P, GROUP_N], f32, tag="h1b")
                    nc.tensor.matmul(out=ph1a[:, :NW], lhsT=w1s_0[:, 0:P],
                                     rhs=xg[:, :NW], start=True, stop=False)
                    nc.tensor.matmul(out=ph1a[:, :NW], lhsT=w1s_1[:, 0:P],
                                     rhs=mean_g[:, :NW], start=False, stop=True)
                    nc.tensor.matmul(out=ph1b[:, :NW], lhsT=w1s_0[:, P:HIDDEN],
                                     rhs=xg[:, :NW], start=True, stop=False)
                    nc.tensor.matmul(out=ph1b[:, :NW], lhsT=w1s_1[:, P:HIDDEN],
                                     rhs=mean_g[:, :NW], start=False, stop=True)
                    h1a = actpool.tile([P, GROUP_N], f32r, tag="h1a_s")
                    h1b = actpool.tile([P, GROUP_N], f32r, tag="h1b_s")
                    nc.scalar.activation(out=h1a[:, :NW], in_=ph1a[:, :NW],
                                         func=Relu, bias=b1s_0[:, 0:1])
                    nc.scalar.activation(out=h1b[:, :NW], in_=ph1b[:, :NW],
                                         func=Relu, bias=b1s_1[:, 0:1])

                    ph2a = pmlp.tile([P, GROUP_N], f32, tag="h2a")
                    ph2b = pmlp.tile([P, GROUP_N], f32, tag="h2b")
                    nc.tensor.matmul(out=ph2a[:, :NW], lhsT=w2s_0[:, 0:P],
                                     rhs=h1a[:, :NW], start=True, stop=False)
                    nc.tensor.matmul(out=ph2a[:, :NW], lhsT=w2s_1[:, 0:P],
                                     rhs=h1b[:, :NW], start=False, stop=True)
                    nc.tensor.matmul(out=ph2b[:, :NW], lhsT=w2s_0[:, P:HIDDEN],
                                     rhs=h1a[:, :NW], start=True, stop=False)
                    nc.tensor.matmul(out=ph2b[:, :NW], lhsT=w2s_1[:, P:HIDDEN],
                                     rhs=h1b[:, :NW], start=False, stop=True)
                    h2a = actpool.tile([P, GROUP_N], f32r, tag="h2a_s")
                    h2b = actpool.tile([P, GROUP_N], f32r, tag="h2b_s")
                    nc.scalar.activation(out=h2a[:, :NW], in_=ph2a[:, :NW],
                                         func=Relu, bias=b2s_0[:, 0:1])
                    nc.scalar.activation(out=h2b[:, :NW], in_=ph2b[:, :NW],
                                         func=Relu, bias=b2s_1[:, 0:1])

                    po = pmlp.tile([P, GROUP_N], f32, tag="o")
                    nc.tensor.matmul(out=po[:, :NW], lhsT=w3s_0[:],
                                     rhs=h2a[:, :NW], start=True, stop=False)
                    nc.tensor.matmul(out=po[:, :NW], lhsT=w3s_1[:],
                                     rhs=h2b[:, :NW], start=False, stop=True)
                    os_ = actpool.tile([P, GROUP_N], f32, tag="out_s")
                    nc.scalar.activation(out=os_[:, :NW], in_=po[:, :NW],
                                         func=Ident, bias=b3s[:, 0:1])
                    nc.sync.dma_start(out=outT_d[:, n0 : n0 + NW], in_=os_[:, :NW])

    # run_bass_via_pjrt (axon path) does not finalize; Bacc needs
    # finalize() to run its compile passes (reg alloc, wait legalization).
    nc.finalize()
    return nc


def _host_prep(x, edge_index, edge_attr):
    """Sort/scale/pad edges; returns (CB, per-core input arrays)."""
    col = np.asarray(edge_index)[1].astype(np.int64)
    x = np.asarray(x, dtype=np.float32)
    counts = np.bincount(col, minlength=N_NODES)
    scale = (1.0 / np.maximum(counts, 1)).astype(np.float32)

    order = np.argsort(col, kind="stable")
    col_s = col[order]
    attr_s = np.asarray(edge_attr, dtype=np.float32)[order]
    attr_s = attr_s * scale[col_s][:, None]

    # per-core, per-window edge counts
    starts = np.empty((N_CORES, WINDOWS + 1), dtype=np.int64)
    for c in range(N_CORES):
        bounds = np.minimum(
            c * NPC_REAL + np.arange(WINDOWS + 1) * W, (c + 1) * NPC_REAL
        )
        starts[c] = np.searchsorted(col_s, bounds)
    cnt = np.diff(starts, axis=1)  # [N_CORES, WINDOWS]

    CB = np.maximum(1, (-(-cnt // P)).max(axis=0)).astype(int)  # ceil, >=1
    offs = np.concatenate([[0], np.cumsum(CB)]).astype(np.int64)
    NCH = int(offs[-1])
    E_pad = NCH * P

    per_core = []
    win_base = np.arange(WINDOWS) * W
    for c in range(N_CORES):
        cnts = cnt[c]
        total = int(cnts.sum())
        src0 = int(starts[c, 0])
        base = np.repeat(offs[:-1] * P, cnts)
        within = np.arange(total) - np.repeat(np.cumsum(cnts) - cnts, cnts)
        edest = base + within

        attr_pad = np.zeros((E_pad, D), np.float32)
        attr_pad[edest] = attr_s[src0 : src0 + total]
        attrT = np.ascontiguousarray(
            attr_pad.reshape(NCH, P, D)
            .transpose(1, 0, 2)
            .reshape(P, NCH * D)
            .astype(np.float16)
        )

        dstrel = np.full((E_pad,), 200.0, np.float32)
        dstrel[edest] = (
            col_s[src0 : src0 + total] - c * NPC_REAL - np.repeat(win_base, cnts)
        )
        dstrelT = np.ascontiguousarray(dstrel.reshape(NCH, P).T)

        xc = np.zeros((NPC, D), np.float32)
        xc[:NPC_REAL] = x[c * NPC_REAL : (c + 1) * NPC_REAL]
        xT = np.ascontiguousarray(xc.T)

        per_core.append({"xT": xT, "attrT": attrT, "dstrelT": dstrelT})
    return tuple(CB.tolist()), per_core


def _build_consts(CB, W1, b1, W2, b2, W3, b3, dstrelT):
    CBmax = max(CB)
    NCH = int(sum(CB))
    CO_B, CO_DST, CONSTW = _const_layout(CBmax, NCH)
    consts = np.zeros((P, CONSTW), np.float32)
    consts[:, CO_B] = b1[:P]
    consts[:, CO_B + 1] = b1[P:]
    consts[:, CO_B + 2] = b2[:P]
    consts[:, CO_B + 3] = b2[P:]
    consts[:, CO_B + 4] = b3
    consts[:, CO_DST : CO_DST + NCH] = dstrelT
    return consts


def _build_wts(W1, W2, W3):
    wts = np.empty((P, 4 * HIDDEN + 2 * DOUT), np.float32)
    wts[:, 0:HIDDEN] = W1[:P]
    wts[:, HIDDEN : 2 * HIDDEN] = W1[P:]
    wts[:, 2 * HIDDEN : 3 * HIDDEN] = W2[:P]
    wts[:, 3 * HIDDEN : 4 * HIDDEN] = W2[P:]
    wts[:, 4 * HIDDEN : 4 * HIDDEN + DOUT] = W3[:P]
    wts[:, 4 * HIDDEN + DOUT : 4 * HIDDEN + 2 * DOUT] = W3[P:]
    return wts


def _build_iota16(CB):
    CBmax = max(CB)
    return np.broadcast_to(
        np.tile(np.arange(W, dtype=np.float16), CBmax)[None, :], (P, CBmax * W)
    ).copy()


def kernel(x, edge_index, edge_attr, W1, b1, W2, b2, W3, b3):
    CB, per_core = _host_prep(x, edge_index, edge_attr)

    key = CB
    if key not in _prog_cache:
        _prog_cache[key] = _build_program(CB)
    nc = _prog_cache[key]

    W1 = np.asarray(W1, np.float32)
    W2 = np.asarray(W2, np.float32)
    W3 = np.asarray(W3, np.float32)
    b1 = np.asarray(b1, np.float32)
    b2 = np.asarray(b2, np.float32)
    b3 = np.asarray(b3, np.float32)
    iota16 = _build_iota16(CB)
    wts = _build_wts(W1, W2, W3)
    in_maps = [
        {
            "xT": pc["xT"],
            "attrT": pc["attrT"],
            "consts": _build_consts(CB, W1, b1, W2, b2, W3, b3, pc["dstrelT"]),
            "iota16": iota16,
            "wts": wts,
        }
        for pc in per_core
    ]

    res = run_bass_kernel_spmd(nc, in_maps, core_ids=list(range(N_CORES)))

    out = np.empty((N_NODES, DOUT), np.float32)
    for c in range(N_CORES):
        out[c * NPC_REAL : (c + 1) * NPC_REAL] = res.results[c]["outT"].T[:NPC_REAL]
    return out


# revision 27
# speedup vs baseline: 1.2438x; 1.2438x over previous
"""NodeNet GNN message-passing kernel for 8 Trainium2 NeuronCores.

Strategy (per sharding hint): shard nodes across the 8 cores; partition
edges by destination node on the host so the scatter-mean is device-local.

Per core (12,500 real nodes, padded to 12,544 = 196 windows of 64 nodes):
  - Host sorts edges by destination, pre-scales each edge row by
    1/count(dst) (so the segment-sum directly yields the mean), pads each
    64-node window's edge list to a multiple of 128, and lays edge
    features out chunk-transposed ([128, nch*128] fp16) so all device
    DMAs are wide and contiguous.
  - Device builds, per 128-edge chunk, a [128 edge, 64 node] fp16 one-hot
    (is_equal of dst-rel against an iota ramp) and uses the TensorEngine:
    meanT[d, n] += matmul(lhsT=attr[e,d], rhs=onehot[e,n]) accumulated in
    PSUM (fp32).  Everything stays feature-major so the 3-layer MLP
    (float32r matmuls) chains without transposes:
    h1T = relu(W1.T @ [xT; meanT] + b1), etc.
  - Output is written feature-major fp16 [128, 12544]; host transposes
    and upcasts.
"""

import numpy as np

import concourse.bacc as bacc
import concourse.mybir as mybir
import concourse.tile as tile
from concourse.bass_utils import run_bass_kernel_spmd

P = 128                    # partitions / matmul contraction tile
D = 128                    # node & edge feature dim
HIDDEN = 256
DOUT = 128
N_NODES = 100000
N_CORES = 8
NPC_REAL = 12500           # real nodes per core
W = 64                     # nodes per binning window
WINDOWS = 196              # windows per core (196*64 = 12544)
NPC = WINDOWS * W          # padded nodes per core
GPW = 8                    # windows per MLP group (512 nodes)
GROUP_N = GPW * W
ATTR_BUFS = 3
OH_BUFS = 4
ACT_BUFS = 2
PBIN_BUFS = 2

_prog_cache: dict = {}

f32 = mybir.dt.float32
f16 = mybir.dt.float16
f32r = mybir.dt.float32r


def _build_program(CB, ablate=()):
    """Build the Bass/Tile program. CB[j] = number of 128-edge chunks for
    window j (identical across cores; per-core data is padded to match).
    ablate: subset of {"mlp", "bin", "oh"} to skip stages (sim studies)."""
    CB = list(CB)
    CBmax = max(CB)
    offs = np.concatenate([[0], np.cumsum(CB)]).astype(int)
    NCH = int(offs[-1])

    nc = bacc.Bacc(None)
    xT_d = nc.dram_tensor("xT", [P, NPC], f16, kind="ExternalInput")
    attrT_d = nc.dram_tensor("attrT", [P, NCH * D], f16, kind="ExternalInput")
    # fp16 consts: iota ramp (CBmax*W) | dstrel (NCH)
    c16_d = nc.dram_tensor("c16", [P, CBmax * W + NCH], f16, kind="ExternalInput")
    # fp32 consts: 5 bias columns
    consts_d = nc.dram_tensor("consts", [P, 5], f32, kind="ExternalInput")
    wts_d = nc.dram_tensor("wts", [P, 4 * HIDDEN + 2 * DOUT], f32r,
                           kind="ExternalInput")
    outT_d = nc.dram_tensor("outT", [P, NPC], f16, kind="ExternalOutput")

    Relu = mybir.ActivationFunctionType.Relu
    Ident = mybir.ActivationFunctionType.Identity

    with tile.TileContext(nc) as tc:
        with (
            tc.tile_pool(name="const", bufs=1) as cpool,
            tc.tile_pool(name="attr", bufs=ATTR_BUFS) as apool,
            tc.tile_pool(name="oh", bufs=OH_BUFS) as ohpool,
            tc.tile_pool(name="acts", bufs=ACT_BUFS) as actpool,
            tc.tile_pool(name="pbin", bufs=PBIN_BUFS, space="PSUM") as pbin,
            tc.tile_pool(name="pmlp", bufs=1, space="PSUM") as pmlp,
        ):
            # --- constants ---
            cs = cpool.tile([P, 5], f32, tag="consts")
            nc.sync.dma_start(out=cs[:], in_=consts_d[:, :])
            ws = cpool.tile([P, 4 * HIDDEN + 2 * DOUT], f32r, tag="wts")
            nc.sync.dma_start(out=ws[:], in_=wts_d[:, :])
            c16 = cpool.tile([P, CBmax * W + NCH], f16, tag="c16")
            nc.sync.dma_start(out=c16[:], in_=c16_d[:, :])
            w1s_0 = ws[:, 0:HIDDEN]
            w1s_1 = ws[:, HIDDEN : 2 * HIDDEN]
            w2s_0 = ws[:, 2 * HIDDEN : 3 * HIDDEN]
            w2s_1 = ws[:, 3 * HIDDEN : 4 * HIDDEN]
            w3s_0 = ws[:, 4 * HIDDEN : 4 * HIDDEN + DOUT]
            w3s_1 = ws[:, 4 * HIDDEN + DOUT : 4 * HIDDEN + 2 * DOUT]
            b1s_0 = cs[:, 0:1]
            b1s_1 = cs[:, 1:2]
            b2s_0 = cs[:, 2:3]
            b2s_1 = cs[:, 3:4]
            b3s = cs[:, 4:5]
            it16 = c16[:, 0 : CBmax * W]
            dstrel_s = c16[:, CBmax * W : CBmax * W + NCH]

            # group sizes: GPW windows each, tapering at the tail to
            # shorten the pipeline drain (last windows are also the
            # smallest thanks to the descending-count permutation)
            gsizes = []
            rem = WINDOWS
            while rem > 2 * GPW:
                gsizes.append(GPW)
                rem -= GPW
            while rem > 0:
                t = max(GPW // 2, min(rem, GPW // 2))
                t = min(t, rem)
                gsizes.append(t)
                rem -= t
            pending_out = []
            gstart = [0]
            for s in gsizes:
                gstart.append(gstart[-1] + s)

            for j in range(WINDOWS):
                cb = CB[j]
                off = int(offs[j])
                g = next(i for i in range(len(gsizes)) if gstart[i + 1] > j)
                sw = j - gstart[g]
                gsz = gsizes[g]

                if sw == 0:
                    # one edge-feature DMA per group of windows
                    goff = off
                    gend = int(offs[gstart[g + 1]])
                    at = apool.tile([P, GPW * CBmax * D], f16, tag="attr")
                    nc.sync.dma_start(
                        out=at[:, : (gend - goff) * D],
                        in_=attrT_d[:, goff * D : gend * D],
                    )
                    while pending_out:
                        pn0, pNW, pos_ = pending_out.pop(0)
                        nc.sync.dma_start(
                            out=outT_d[:, pn0 : pn0 + pNW], in_=pos_[:, :pNW]
                        )
                woff = off - goff  # window's chunk offset within group tile

                oh = ohpool.tile([P, CBmax * W], f16, tag="oh")
                if "oh" not in ablate:
                    nc.vector.tensor_tensor(
                        out=oh[:, : cb * W].rearrange("p (c m) -> p c m", m=W),
                        in0=dstrel_s[:, off : off + cb].to_broadcast([P, cb, W]),
                        in1=it16[:, : cb * W].rearrange("p (c m) -> p c m", m=W),
                        op=mybir.AluOpType.is_equal,
                    )

                pm = pbin.tile([P, W], f32, tag="mean")
                for ch in range(cb if "bin" not in ablate else 0):
                    nc.tensor.matmul(
                        out=pm[:],
                        lhsT=at[:, (woff + ch) * D : (woff + ch + 1) * D],
                        rhs=oh[:, ch * W : (ch + 1) * W],
                        start=(ch == 0),
                        stop=(ch == cb - 1),
                    )

                if sw == 0:
                    mean_g = actpool.tile([P, GROUP_N], f32r, tag="mean_g")
                if "bin" not in ablate:
                    nc.scalar.copy(out=mean_g[:, sw * W : (sw + 1) * W], in_=pm[:])

                if ("mlp" not in ablate) and (sw == gsz - 1):
                    # --- MLP over this group of nodes (feature-major) ---
                    NW = gsz * W
                    n0 = gstart[g] * W
                    xg16 = actpool.tile([P, GROUP_N], f16, tag="xg16")
                    nc.sync.dma_start(out=xg16[:, :NW], in_=xT_d[:, n0 : n0 + NW])
                    xg = actpool.tile([P, GROUP_N], f32r, tag="xg")
                    nc.vector.tensor_copy(out=xg[:, :NW], in_=xg16[:, :NW])

                    ph1a = pmlp.tile([P, GROUP_N], f32, tag="h1a")
                    ph1b = pmlp.tile([P, GROUP_N], f32, tag="h1b")
                    nc.tensor.matmul(out=ph1a[:, :NW], lhsT=w1s_0[:, 0:P],
                                     rhs=xg[:, :NW], start=True, stop=False)
                    nc.tensor.matmul(out=ph1a[:, :NW], lhsT=w1s_1[:, 0:P],
                                     rhs=mean_g[:, :NW], start=False, stop=True)
                    nc.tensor.matmul(out=ph1b[:, :NW], lhsT=w1s_0[:, P:HIDDEN],
                                     rhs=xg[:, :NW], start=True, stop=False)
                    nc.tensor.matmul(out=ph1b[:, :NW], lhsT=w1s_1[:, P:HIDDEN],
                                     rhs=mean_g[:, :NW], start=False, stop=True)
                    h1a = actpool.tile([P, GROUP_N], f32r, tag="h1a_s")
                    h1b = actpool.tile([P, GROUP_N], f32r, tag="h1b_s")
                    nc.scalar.activation(out=h1a[:, :NW], in_=ph1a[:, :NW],
                                         func=Relu, bias=b1s_0[:, 0:1])
                    nc.scalar.activation(out=h1b[:, :NW], in_=ph1b[:, :NW],
                                         func=Relu, bias=b1s_1[:, 0:1])

                    ph2a = pmlp.tile([P, GROUP_N], f32, tag="h2a")
                    ph2b = pmlp.tile([P, GROUP_N], f32, tag="h2b")
                    nc.tensor.matmul(out=ph2a[:, :NW], lhsT=w2s_0[:, 0:P],
                                     rhs=h1a[:, :NW], start=True, stop=False)
                    nc.tensor.matmul(out=ph2a[:, :NW], lhsT=w2s_1[:, 0:P],
                                     rhs=h1b[:, :NW], start=False, stop=True)
                    nc.tensor.matmul(out=ph2b[:, :NW], lhsT=w2s_0[:, P:HIDDEN],
                                     rhs=h1a[:, :NW], start=True, stop=False)
                    nc.tensor.matmul(out=ph2b[:, :NW], lhsT=w2s_1[:, P:HIDDEN],
                                     rhs=h1b[:, :NW], start=False, stop=True)
                    h2a = actpool.tile([P, GROUP_N], f32r, tag="h2a_s")
                    h2b = actpool.tile([P, GROUP_N], f32r, tag="h2b_s")
                    nc.scalar.activation(out=h2a[:, :NW], in_=ph2a[:, :NW],
                                         func=Relu, bias=b2s_0[:, 0:1])
                    nc.scalar.activation(out=h2b[:, :NW], in_=ph2b[:, :NW],
                                         func=Relu, bias=b2s_1[:, 0:1])

                    po = pmlp.tile([P, GROUP_N], f32, tag="o")
                    nc.tensor.matmul(out=po[:, :NW], lhsT=w3s_0[:],
                                     rhs=h2a[:, :NW], start=True, stop=False)
                    nc.tensor.matmul(out=po[:, :NW], lhsT=w3s_1[:],
                                     rhs=h2b[:, :NW], start=False, stop=True)
                    os_ = actpool.tile([P, GROUP_N], f16, tag="out_s")
                    nc.scalar.activation(out=os_[:, :NW], in_=po[:, :NW],
                                         func=Ident, bias=b3s[:, 0:1])
                    # defer the store one group so SP issues the next
                    # group's prefetches before blocking on this os_ tile
                    pending_out.append((n0, NW, os_))

            while pending_out:
                pn0, pNW, pos_ = pending_out.pop(0)
                nc.sync.dma_start(out=outT_d[:, pn0 : pn0 + pNW], in_=pos_[:, :pNW])

    # run_bass_via_pjrt (axon path) does not finalize; Bacc needs
    # finalize() to run its compile passes (reg alloc, wait legalization).
    nc.finalize()
    return nc


def _host_prep(x, edge_index, edge_attr):
    """Sort/scale/pad edges; returns (CB, per-core input arrays)."""
    col = np.asarray(edge_index)[1].astype(np.int64)
    x = np.asarray(x, dtype=np.float32)
    counts = np.bincount(col, minlength=N_NODES)
    scale = (1.0 / np.maximum(counts, 1)).astype(np.float32)

    order = np.argsort(col, kind="stable")
    col_s = col[order]
    attr_s = np.asarray(edge_attr, dtype=np.float32)[order]
    attr_s = attr_s * scale[col_s][:, None]

    # per-core, per-window edge counts
    starts = np.empty((N_CORES, WINDOWS + 1), dtype=np.int64)
    for c in range(N_CORES):
        bounds = np.minimum(
            c * NPC_REAL + np.arange(WINDOWS + 1) * W, (c + 1) * NPC_REAL
        )
        starts[c] = np.searchsorted(col_s, bounds)
    cnt = np.diff(starts, axis=1)  # [N_CORES, WINDOWS]

    # Each core processes its windows sorted by descending edge count.
    # Window slot j then holds every core's j-th order statistic, so the
    # cross-core max (CB must be shared, the program is SPMD) wastes far
    # less padding than positional assignment.  Small windows land last,
    # which also shortens the pipeline drain.  Host un-permutes outputs.
    order = np.argsort(-cnt, axis=1, kind="stable")  # [N_CORES, WINDOWS]
    cnt_s = np.take_along_axis(cnt, order, axis=1)

    CB = np.maximum(1, (-(-cnt_s // P)).max(axis=0)).astype(int)  # ceil, >=1
    offs = np.concatenate([[0], np.cumsum(CB)]).astype(np.int64)
    NCH = int(offs[-1])
    E_pad = NCH * P

    per_core = []
    for c in range(N_CORES):
        ordc = order[c]
        cnts = cnt_s[c]                      # counts in processing order
        total = int(cnts.sum())
        # edge source rows (into col_s/attr_s), in processing order
        src_idx = np.concatenate(
            [np.arange(starts[c, w], starts[c, w + 1]) for w in ordc]
        )
        base = np.repeat(offs[:-1] * P, cnts)
        within = np.arange(total) - np.repeat(np.cumsum(cnts) - cnts, cnts)
        edest = base + within

        attr_pad = np.zeros((E_pad, D), np.float32)
        attr_pad[edest] = attr_s[src_idx]
        attrT = np.ascontiguousarray(
            attr_pad.reshape(NCH, P, D)
            .transpose(1, 0, 2)
            .reshape(P, NCH * D)
            .astype(np.float16)
        )

        # dst relative to the processed window's node base
        win_base_proc = c * NPC_REAL + ordc * W  # global node base per slot
        dstrel = np.full((E_pad,), 200.0, np.float16)
        dstrel[edest] = (
            col_s[src_idx] - np.repeat(win_base_proc, cnts)
        ).astype(np.float16)
        dstrelT = np.ascontiguousarray(dstrel.reshape(NCH, P).T)

        # node features per 64-node window slot, zero-padded per slot
        xc = np.zeros((WINDOWS, W, D), np.float16)
        for j, w in enumerate(ordc):
            n0 = c * NPC_REAL + w * W
            n1 = min(n0 + W, (c + 1) * NPC_REAL)
            xc[j, : n1 - n0] = x[n0:n1].astype(np.float16)
        xT = np.ascontiguousarray(xc.reshape(NPC, D).T)

        per_core.append(
            {"xT": xT, "attrT": attrT, "dstrelT": dstrelT, "order": ordc}
        )
    return tuple(CB.tolist()), per_core


def _build_consts(b1, b2, b3):
    consts = np.zeros((P, 5), np.float32)
    consts[:, 0] = b1[:P]
    consts[:, 1] = b1[P:]
    consts[:, 2] = b2[:P]
    consts[:, 3] = b2[P:]
    consts[:, 4] = b3
    return consts


def _build_wts(W1, W2, W3):
    wts = np.empty((P, 4 * HIDDEN + 2 * DOUT), np.float32)
    wts[:, 0:HIDDEN] = W1[:P]
    wts[:, HIDDEN : 2 * HIDDEN] = W1[P:]
    wts[:, 2 * HIDDEN : 3 * HIDDEN] = W2[:P]
    wts[:, 3 * HIDDEN : 4 * HIDDEN] = W2[P:]
    wts[:, 4 * HIDDEN : 4 * HIDDEN + DOUT] = W3[:P]
    wts[:, 4 * HIDDEN + DOUT : 4 * HIDDEN + 2 * DOUT] = W3[P:]
    return wts


def _build_c16(CB, dstrelT):
    """fp16 consts row-block: iota ramp | dstrel."""
    CBmax = max(CB)
    NCH = int(sum(CB))
    c16 = np.empty((P, CBmax * W + NCH), np.float16)
    c16[:, 0 : CBmax * W] = np.tile(np.arange(W, dtype=np.float16), CBmax)[None, :]
    c16[:, CBmax * W :] = dstrelT
    return c16


def kernel(x, edge_index, edge_attr, W1, b1, W2, b2, W3, b3):
    CB, per_core = _host_prep(x, edge_index, edge_attr)

    key = CB
    if key not in _prog_cache:
        _prog_cache[key] = _build_program(CB)
    nc = _prog_cache[key]

    W1 = np.asarray(W1, np.float32)
    W2 = np.asarray(W2, np.float32)
    W3 = np.asarray(W3, np.float32)
    b1 = np.asarray(b1, np.float32)
    b2 = np.asarray(b2, np.float32)
    b3 = np.asarray(b3, np.float32)
    consts = _build_consts(b1, b2, b3)
    wts = _build_wts(W1, W2, W3)
    in_maps = [
        {
            "xT": pc["xT"],
            "attrT": pc["attrT"],
            "c16": _build_c16(CB, pc["dstrelT"]),
            "consts": consts,
            "wts": wts,
        }
        for pc in per_core
    ]

    res = run_bass_kernel_spmd(nc, in_maps, core_ids=list(range(N_CORES)))

    out = np.empty((N_NODES, DOUT), np.float32)
    for c in range(N_CORES):
        o = res.results[c]["outT"].T.astype(np.float32).reshape(WINDOWS, W, DOUT)
        for j, w in enumerate(per_core[c]["order"]):
            n0 = c * NPC_REAL + int(w) * W
            n1 = min(n0 + W, (c + 1) * NPC_REAL)
            out[n0:n1] = o[j, : n1 - n0]
    return out


# revision 33
# speedup vs baseline: 1.2655x; 1.0175x over previous
"""NodeNet GNN message-passing kernel for 8 Trainium2 NeuronCores.

Strategy (per sharding hint): shard nodes across the 8 cores; partition
edges by destination node on the host so the scatter-mean is device-local.

Per core (12,500 real nodes, padded to 12,544 = 196 windows of 64 nodes):
  - Host sorts edges by destination, pre-scales each edge row by
    1/count(dst) (so the segment-sum directly yields the mean), pads each
    64-node window's edge list to a multiple of 128, and lays edge
    features out chunk-transposed ([128, nch*128] fp16) so all device
    DMAs are wide and contiguous.
  - Device builds, per 128-edge chunk, a [128 edge, 64 node] fp16 one-hot
    (is_equal of dst-rel against an iota ramp) and uses the TensorEngine:
    meanT[d, n] += matmul(lhsT=attr[e,d], rhs=onehot[e,n]) accumulated in
    PSUM (fp32).  Everything stays feature-major so the 3-layer MLP
    (float32r matmuls) chains without transposes:
    h1T = relu(W1.T @ [xT; meanT] + b1), etc.
  - Output is written feature-major fp16 [128, 12544]; host transposes
    and upcasts.
"""

import numpy as np

import concourse.bacc as bacc
import concourse.mybir as mybir
import concourse.tile as tile
from concourse.bass_utils import run_bass_kernel_spmd

P = 128                    # partitions / matmul contraction tile
D = 128                    # node & edge feature dim
HIDDEN = 256
DOUT = 128
N_NODES = 100000
N_CORES = 8
NPC_REAL = 12500           # real nodes per core
W = 64                     # nodes per binning window
WINDOWS = 196              # windows per core (196*64 = 12544)
NPC = WINDOWS * W          # padded nodes per core
GPW = 8                    # windows per MLP group (512 nodes)
GROUP_N = GPW * W
ATTR_BUFS = 3
OH_BUFS = 4
ACT_BUFS = 3
PBIN_BUFS = 3

_prog_cache: dict = {}

f32 = mybir.dt.float32
f16 = mybir.dt.float16
f32r = mybir.dt.float32r


def _build_program(CB, ablate=()):
    """Build the Bass/Tile program. CB[j] = number of 128-edge chunks for
    window j (identical across cores; per-core data is padded to match).
    ablate: subset of {"mlp", "bin", "oh"} to skip stages (sim studies)."""
    CB = list(CB)
    CBmax = max(CB)
    offs = np.concatenate([[0], np.cumsum(CB)]).astype(int)
    NCH = int(offs[-1])

    nc = bacc.Bacc(None)
    xT_d = nc.dram_tensor("xT", [P, NPC], f16, kind="ExternalInput")
    attrT_d = nc.dram_tensor("attrT", [P, NCH * D], f16, kind="ExternalInput")
    # fp16 consts: iota ramp (CBmax*W) | dstrel (NCH)
    c16_d = nc.dram_tensor("c16", [P, CBmax * W + NCH], f16, kind="ExternalInput")
    # fp32 consts: 5 bias columns
    consts_d = nc.dram_tensor("consts", [P, 5], f32, kind="ExternalInput")
    wts_d = nc.dram_tensor("wts", [P, 4 * HIDDEN + 2 * DOUT], f32r,
                           kind="ExternalInput")
    outT_d = nc.dram_tensor("outT", [P, NPC], f16, kind="ExternalOutput")

    Relu = mybir.ActivationFunctionType.Relu
    Ident = mybir.ActivationFunctionType.Identity

    with tile.TileContext(nc) as tc:
        with (
            tc.tile_pool(name="const", bufs=1) as cpool,
            tc.tile_pool(name="attr", bufs=ATTR_BUFS) as apool,
            tc.tile_pool(name="oh", bufs=OH_BUFS) as ohpool,
            tc.tile_pool(name="acts", bufs=ACT_BUFS) as actpool,
            tc.tile_pool(name="pbin", bufs=PBIN_BUFS, space="PSUM") as pbin,
            tc.tile_pool(name="pmlp", bufs=1, space="PSUM") as pmlp,
        ):
            # --- constants ---
            cs = cpool.tile([P, 5], f32, tag="consts")
            nc.sync.dma_start(out=cs[:], in_=consts_d[:, :])
            ws = cpool.tile([P, 4 * HIDDEN + 2 * DOUT], f32r, tag="wts")
            nc.sync.dma_start(out=ws[:], in_=wts_d[:, :])
            c16 = cpool.tile([P, CBmax * W + NCH], f16, tag="c16")
            nc.sync.dma_start(out=c16[:], in_=c16_d[:, :])
            w1s_0 = ws[:, 0:HIDDEN]
            w1s_1 = ws[:, HIDDEN : 2 * HIDDEN]
            w2s_0 = ws[:, 2 * HIDDEN : 3 * HIDDEN]
            w2s_1 = ws[:, 3 * HIDDEN : 4 * HIDDEN]
            w3s_0 = ws[:, 4 * HIDDEN : 4 * HIDDEN + DOUT]
            w3s_1 = ws[:, 4 * HIDDEN + DOUT : 4 * HIDDEN + 2 * DOUT]
            b1s_0 = cs[:, 0:1]
            b1s_1 = cs[:, 1:2]
            b2s_0 = cs[:, 2:3]
            b2s_1 = cs[:, 3:4]
            b3s = cs[:, 4:5]
            it16 = c16[:, 0 : CBmax * W]
            dstrel_s = c16[:, CBmax * W : CBmax * W + NCH]

            # group sizes: GPW windows each, tapering at the tail to
            # shorten the pipeline drain (last windows are also the
            # smallest thanks to the descending-count permutation)
            gsizes = []
            rem = WINDOWS
            while rem > 2 * GPW:
                gsizes.append(GPW)
                rem -= GPW
            while rem > 0:
                t = max(GPW // 2, min(rem, GPW // 2))
                t = min(t, rem)
                gsizes.append(t)
                rem -= t
            pending_out = []
            gstart = [0]
            for s in gsizes:
                gstart.append(gstart[-1] + s)

            for j in range(WINDOWS):
                cb = CB[j]
                off = int(offs[j])
                g = next(i for i in range(len(gsizes)) if gstart[i + 1] > j)
                sw = j - gstart[g]
                gsz = gsizes[g]

                if sw == 0:
                    # one edge-feature DMA per group of windows
                    goff = off
                    gend = int(offs[gstart[g + 1]])
                    at = apool.tile([P, GPW * CBmax * D], f16, tag="attr")
                    nc.sync.dma_start(
                        out=at[:, : (gend - goff) * D],
                        in_=attrT_d[:, goff * D : gend * D],
                    )
                    while pending_out:
                        pn0, pNW, pos_ = pending_out.pop(0)
                        nc.sync.dma_start(
                            out=outT_d[:, pn0 : pn0 + pNW], in_=pos_[:, :pNW]
                        )
                woff = off - goff  # window's chunk offset within group tile

                oh = ohpool.tile([P, CBmax * W], f16, tag="oh")
                if "oh" not in ablate:
                    nc.vector.tensor_tensor(
                        out=oh[:, : cb * W].rearrange("p (c m) -> p c m", m=W),
                        in0=dstrel_s[:, off : off + cb].to_broadcast([P, cb, W]),
                        in1=it16[:, : cb * W].rearrange("p (c m) -> p c m", m=W),
                        op=mybir.AluOpType.is_equal,
                    )

                pm = pbin.tile([P, W], f32, tag="mean")
                for ch in range(cb if "bin" not in ablate else 0):
                    nc.tensor.matmul(
                        out=pm[:],
                        lhsT=at[:, (woff + ch) * D : (woff + ch + 1) * D],
                        rhs=oh[:, ch * W : (ch + 1) * W],
                        start=(ch == 0),
                        stop=(ch == cb - 1),
                    )

                if sw == 0:
                    mean_g = actpool.tile([P, GROUP_N], f32r, tag="mean_g")
                if "bin" not in ablate:
                    nc.scalar.copy(out=mean_g[:, sw * W : (sw + 1) * W], in_=pm[:])

                if ("mlp" not in ablate) and (sw == gsz - 1):
                    # --- MLP over this group of nodes (feature-major) ---
                    NW = gsz * W
                    n0 = gstart[g] * W
                    xg16 = actpool.tile([P, GROUP_N], f16, tag="xg16")
                    nc.sync.dma_start(out=xg16[:, :NW], in_=xT_d[:, n0 : n0 + NW])
                    xg = actpool.tile([P, GROUP_N], f32r, tag="xg")
                    nc.vector.tensor_copy(out=xg[:, :NW], in_=xg16[:, :NW])

                    ph1a = pmlp.tile([P, GROUP_N], f32, tag="h1a")
                    ph1b = pmlp.tile([P, GROUP_N], f32, tag="h1b")
                    nc.tensor.matmul(out=ph1a[:, :NW], lhsT=w1s_0[:, 0:P],
                                     rhs=xg[:, :NW], start=True, stop=False)
                    nc.tensor.matmul(out=ph1a[:, :NW], lhsT=w1s_1[:, 0:P],
                                     rhs=mean_g[:, :NW], start=False, stop=True)
                    nc.tensor.matmul(out=ph1b[:, :NW], lhsT=w1s_0[:, P:HIDDEN],
                                     rhs=xg[:, :NW], start=True, stop=False)
                    nc.tensor.matmul(out=ph1b[:, :NW], lhsT=w1s_1[:, P:HIDDEN],
                                     rhs=mean_g[:, :NW], start=False, stop=True)
                    h1a = actpool.tile([P, GROUP_N], f32r, tag="h1a_s")
                    h1b = actpool.tile([P, GROUP_N], f32r, tag="h1b_s")
                    nc.scalar.activation(out=h1a[:, :NW], in_=ph1a[:, :NW],
                                         func=Relu, bias=b1s_0[:, 0:1])
                    nc.scalar.activation(out=h1b[:, :NW], in_=ph1b[:, :NW],
                                         func=Relu, bias=b1s_1[:, 0:1])

                    ph2a = pmlp.tile([P, GROUP_N], f32, tag="h2a")
                    ph2b = pmlp.tile([P, GROUP_N], f32, tag="h2b")
                    nc.tensor.matmul(out=ph2a[:, :NW], lhsT=w2s_0[:, 0:P],
                                     rhs=h1a[:, :NW], start=True, stop=False)
                    nc.tensor.matmul(out=ph2a[:, :NW], lhsT=w2s_1[:, 0:P],
                                     rhs=h1b[:, :NW], start=False, stop=True)
                    nc.tensor.matmul(out=ph2b[:, :NW], lhsT=w2s_0[:, P:HIDDEN],
                                     rhs=h1a[:, :NW], start=True, stop=False)
                    nc.tensor.matmul(out=ph2b[:, :NW], lhsT=w2s_1[:, P:HIDDEN],
                                     rhs=h1b[:, :NW], start=False, stop=True)
                    h2a = actpool.tile([P, GROUP_N], f32r, tag="h2a_s")
                    h2b = actpool.tile([P, GROUP_N], f32r, tag="h2b_s")
                    nc.scalar.activation(out=h2a[:, :NW], in_=ph2a[:, :NW],
                                         func=Relu, bias=b2s_0[:, 0:1])
                    nc.scalar.activation(out=h2b[:, :NW], in_=ph2b[:, :NW],
                                         func=Relu, bias=b2s_1[:, 0:1])

                    po = pmlp.tile([P, GROUP_N], f32, tag="o")
                    nc.tensor.matmul(out=po[:, :NW], lhsT=w3s_0[:],
                                     rhs=h2a[:, :NW], start=True, stop=False)
                    nc.tensor.matmul(out=po[:, :NW], lhsT=w3s_1[:],
                                     rhs=h2b[:, :NW], start=False, stop=True)
                    os_ = actpool.tile([P, GROUP_N], f16, tag="out_s")
                    nc.scalar.activation(out=os_[:, :NW], in_=po[:, :NW],
                                         func=Ident, bias=b3s[:, 0:1])
                    # defer the store one group so SP issues the next
                    # group's prefetches before blocking on this os_ tile
                    pending_out.append((n0, NW, os_))

            while pending_out:
                pn0, pNW, pos_ = pending_out.pop(0)
                nc.sync.dma_start(out=outT_d[:, pn0 : pn0 + pNW], in_=pos_[:, :pNW])

    # run_bass_via_pjrt (axon path) does not finalize; Bacc needs
    # finalize() to run its compile passes (reg alloc, wait legalization).
    nc.finalize()
    return nc


def _host_prep(x, edge_index, edge_attr):
    """Sort/scale/pad edges; returns (CB, per-core input arrays)."""
    col = np.asarray(edge_index)[1].astype(np.int64)
    x = np.asarray(x, dtype=np.float32)
    counts = np.bincount(col, minlength=N_NODES)
    scale = (1.0 / np.maximum(counts, 1)).astype(np.float32)

    order = np.argsort(col, kind="stable")
    col_s = col[order]
    attr_s = np.asarray(edge_attr, dtype=np.float32)[order]
    attr_s = attr_s * scale[col_s][:, None]

    # per-core, per-window edge counts
    starts = np.empty((N_CORES, WINDOWS + 1), dtype=np.int64)
    for c in range(N_CORES):
        bounds = np.minimum(
            c * NPC_REAL + np.arange(WINDOWS + 1) * W, (c + 1) * NPC_REAL
        )
        starts[c] = np.searchsorted(col_s, bounds)
    cnt = np.diff(starts, axis=1)  # [N_CORES, WINDOWS]

    # Each core processes its windows sorted by descending edge count.
    # Window slot j then holds every core's j-th order statistic, so the
    # cross-core max (CB must be shared, the program is SPMD) wastes far
    # less padding than positional assignment.  Small windows land last,
    # which also shortens the pipeline drain.  Host un-permutes outputs.
    order = np.argsort(-cnt, axis=1, kind="stable")  # [N_CORES, WINDOWS]
    cnt_s = np.take_along_axis(cnt, order, axis=1)

    CB = np.maximum(1, (-(-cnt_s // P)).max(axis=0)).astype(int)  # ceil, >=1
    offs = np.concatenate([[0], np.cumsum(CB)]).astype(np.int64)
    NCH = int(offs[-1])
    E_pad = NCH * P

    per_core = []
    for c in range(N_CORES):
        ordc = order[c]
        cnts = cnt_s[c]                      # counts in processing order
        total = int(cnts.sum())
        # edge source rows (into col_s/attr_s), in processing order
        src_idx = np.concatenate(
            [np.arange(starts[c, w], starts[c, w + 1]) for w in ordc]
        )
        base = np.repeat(offs[:-1] * P, cnts)
        within = np.arange(total) - np.repeat(np.cumsum(cnts) - cnts, cnts)
        edest = base + within

        attr_pad = np.zeros((E_pad, D), np.float32)
        attr_pad[edest] = attr_s[src_idx]
        attrT = np.ascontiguousarray(
            attr_pad.reshape(NCH, P, D)
            .transpose(1, 0, 2)
            .reshape(P, NCH * D)
            .astype(np.float16)
        )

        # dst relative to the processed window's node base
        win_base_proc = c * NPC_REAL + ordc * W  # global node base per slot
        dstrel = np.full((E_pad,), 200.0, np.float16)
        dstrel[edest] = (
            col_s[src_idx] - np.repeat(win_base_proc, cnts)
        ).astype(np.float16)
        dstrelT = np.ascontiguousarray(dstrel.reshape(NCH, P).T)

        # node features per 64-node window slot, zero-padded per slot
        xc = np.zeros((WINDOWS, W, D), np.float16)
        for j, w in enumerate(ordc):
            n0 = c * NPC_REAL + w * W
            n1 = min(n0 + W, (c + 1) * NPC_REAL)
            xc[j, : n1 - n0] = x[n0:n1].astype(np.float16)
        xT = np.ascontiguousarray(xc.reshape(NPC, D).T)

        per_core.append(
            {"xT": xT, "attrT": attrT, "dstrelT": dstrelT, "order": ordc}
        )
    return tuple(CB.tolist()), per_core


def _build_consts(b1, b2, b3):
    consts = np.zeros((P, 5), np.float32)
    consts[:, 0] = b1[:P]
    consts[:, 1] = b1[P:]
    consts[:, 2] = b2[:P]
    consts[:, 3] = b2[P:]
    consts[:, 4] = b3
    return consts


def _build_wts(W1, W2, W3):
    wts = np.empty((P, 4 * HIDDEN + 2 * DOUT), np.float32)
    wts[:, 0:HIDDEN] = W1[:P]
    wts[:, HIDDEN : 2 * HIDDEN] = W1[P:]
    wts[:, 2 * HIDDEN : 3 * HIDDEN] = W2[:P]
    wts[:, 3 * HIDDEN : 4 * HIDDEN] = W2[P:]
    wts[:, 4 * HIDDEN : 4 * HIDDEN + DOUT] = W3[:P]
    wts[:, 4 * HIDDEN + DOUT : 4 * HIDDEN + 2 * DOUT] = W3[P:]
    return wts


def _build_c16(CB, dstrelT):
    """fp16 consts row-block: iota ramp | dstrel."""
    CBmax = max(CB)
    NCH = int(sum(CB))
    c16 = np.empty((P, CBmax * W + NCH), np.float16)
    c16[:, 0 : CBmax * W] = np.tile(np.arange(W, dtype=np.float16), CBmax)[None, :]
    c16[:, CBmax * W :] = dstrelT
    return c16


def kernel(x, edge_index, edge_attr, W1, b1, W2, b2, W3, b3):
    CB, per_core = _host_prep(x, edge_index, edge_attr)

    key = CB
    if key not in _prog_cache:
        _prog_cache[key] = _build_program(CB)
    nc = _prog_cache[key]

    W1 = np.asarray(W1, np.float32)
    W2 = np.asarray(W2, np.float32)
    W3 = np.asarray(W3, np.float32)
    b1 = np.asarray(b1, np.float32)
    b2 = np.asarray(b2, np.float32)
    b3 = np.asarray(b3, np.float32)
    consts = _build_consts(b1, b2, b3)
    wts = _build_wts(W1, W2, W3)
    in_maps = [
        {
            "xT": pc["xT"],
            "attrT": pc["attrT"],
            "c16": _build_c16(CB, pc["dstrelT"]),
            "consts": consts,
            "wts": wts,
        }
        for pc in per_core
    ]

    res = run_bass_kernel_spmd(nc, in_maps, core_ids=list(range(N_CORES)))

    out = np.empty((N_NODES, DOUT), np.float32)
    for c in range(N_CORES):
        o = res.results[c]["outT"].T.astype(np.float32).reshape(WINDOWS, W, DOUT)
        for j, w in enumerate(per_core[c]["order"]):
            n0 = c * NPC_REAL + int(w) * W
            n1 = min(n0 + W, (c + 1) * NPC_REAL)
            out[n0:n1] = o[j, : n1 - n0]
    return out


# revision 34
# speedup vs baseline: 1.2681x; 1.0020x over previous
"""NodeNet GNN message-passing kernel for 8 Trainium2 NeuronCores.

Strategy (per sharding hint): shard nodes across the 8 cores; partition
edges by destination node on the host so the scatter-mean is device-local.

Per core (12,500 real nodes, padded to 12,544 = 196 windows of 64 nodes):
  - Host sorts edges by destination, pre-scales each edge row by
    1/count(dst) (so the segment-sum directly yields the mean), pads each
    64-node window's edge list to a multiple of 128, and lays edge
    features out chunk-transposed ([128, nch*128] fp16) so all device
    DMAs are wide and contiguous.
  - Device builds, per 128-edge chunk, a [128 edge, 64 node] fp16 one-hot
    (is_equal of dst-rel against an iota ramp) and uses the TensorEngine:
    meanT[d, n] += matmul(lhsT=attr[e,d], rhs=onehot[e,n]) accumulated in
    PSUM (fp32).  Everything stays feature-major so the 3-layer MLP
    (float32r matmuls) chains without transposes:
    h1T = relu(W1.T @ [xT; meanT] + b1), etc.
  - Output is written feature-major fp16 [128, 12544]; host transposes
    and upcasts.
"""

import numpy as np

import concourse.bacc as bacc
import concourse.mybir as mybir
import concourse.tile as tile
from concourse.bass_utils import run_bass_kernel_spmd

P = 128                    # partitions / matmul contraction tile
D = 128                    # node & edge feature dim
HIDDEN = 256
DOUT = 128
N_NODES = 100000
N_CORES = 8
NPC_REAL = 12500           # real nodes per core
W = 64                     # nodes per binning window
WINDOWS = 196              # windows per core (196*64 = 12544)
NPC = WINDOWS * W          # padded nodes per core
GPW = 8                    # windows per MLP group (512 nodes)
GROUP_N = GPW * W
ATTR_BUFS = 3
OH_BUFS = 4
ACT_BUFS = 3
PBIN_BUFS = 3

_prog_cache: dict = {}

f32 = mybir.dt.float32
f16 = mybir.dt.float16
f32r = mybir.dt.float32r


def _build_program(CB, ablate=()):
    """Build the Bass/Tile program. CB[j] = number of 128-edge chunks for
    window j (identical across cores; per-core data is padded to match).
    ablate: subset of {"mlp", "bin", "oh"} to skip stages (sim studies)."""
    CB = list(CB)
    CBmax = max(CB)
    offs = np.concatenate([[0], np.cumsum(CB)]).astype(int)
    NCH = int(offs[-1])

    nc = bacc.Bacc(None)
    xT_d = nc.dram_tensor("xT", [P, NPC], f16, kind="ExternalInput")
    attrT_d = nc.dram_tensor("attrT", [P, NCH * D], f16, kind="ExternalInput")
    # fp16 consts: iota ramp (CBmax*W) | dstrel (NCH)
    c16_d = nc.dram_tensor("c16", [P, CBmax * W + NCH], f16, kind="ExternalInput")
    # fp32 consts: 5 bias columns
    consts_d = nc.dram_tensor("consts", [P, 5], f32, kind="ExternalInput")
    wts_d = nc.dram_tensor("wts", [P, 4 * HIDDEN + 2 * DOUT], f32r,
                           kind="ExternalInput")
    outT_d = nc.dram_tensor("outT", [P, NPC], f16, kind="ExternalOutput")

    Relu = mybir.ActivationFunctionType.Relu
    Ident = mybir.ActivationFunctionType.Identity

    with tile.TileContext(nc) as tc:
        with (
            tc.tile_pool(name="const", bufs=1) as cpool,
            tc.tile_pool(name="attr", bufs=ATTR_BUFS) as apool,
            tc.tile_pool(name="oh", bufs=OH_BUFS) as ohpool,
            tc.tile_pool(name="acts", bufs=ACT_BUFS) as actpool,
            tc.tile_pool(name="pbin", bufs=PBIN_BUFS, space="PSUM") as pbin,
            tc.tile_pool(name="pmlp", bufs=1, space="PSUM") as pmlp,
        ):
            # --- constants (tiles now; DMAs after the first attr DMA so
            # the edge stream starts immediately) ---
            cs = cpool.tile([P, 5], f32, tag="consts")
            ws = cpool.tile([P, 4 * HIDDEN + 2 * DOUT], f32r, tag="wts")
            c16 = cpool.tile([P, CBmax * W + NCH], f16, tag="c16")
            w1s_0 = ws[:, 0:HIDDEN]
            w1s_1 = ws[:, HIDDEN : 2 * HIDDEN]
            w2s_0 = ws[:, 2 * HIDDEN : 3 * HIDDEN]
            w2s_1 = ws[:, 3 * HIDDEN : 4 * HIDDEN]
            w3s_0 = ws[:, 4 * HIDDEN : 4 * HIDDEN + DOUT]
            w3s_1 = ws[:, 4 * HIDDEN + DOUT : 4 * HIDDEN + 2 * DOUT]
            b1s_0 = cs[:, 0:1]
            b1s_1 = cs[:, 1:2]
            b2s_0 = cs[:, 2:3]
            b2s_1 = cs[:, 3:4]
            b3s = cs[:, 4:5]
            it16 = c16[:, 0 : CBmax * W]
            dstrel_s = c16[:, CBmax * W : CBmax * W + NCH]

            # group sizes: GPW windows each, tapering at the tail to
            # shorten the pipeline drain (last windows are also the
            # smallest thanks to the descending-count permutation)
            gsizes = []
            rem = WINDOWS
            while rem > 2 * GPW:
                gsizes.append(GPW)
                rem -= GPW
            for t in (GPW // 2, GPW // 2, GPW // 4, GPW // 4):
                t = min(t, rem)
                if t > 0:
                    gsizes.append(t)
                    rem -= t
            while rem > 0:
                t = min(GPW // 4, rem)
                gsizes.append(t)
                rem -= t
            pending_out = []
            gstart = [0]
            for s in gsizes:
                gstart.append(gstart[-1] + s)

            for j in range(WINDOWS):
                cb = CB[j]
                off = int(offs[j])
                g = next(i for i in range(len(gsizes)) if gstart[i + 1] > j)
                sw = j - gstart[g]
                gsz = gsizes[g]

                if sw == 0:
                    # one edge-feature DMA per group of windows
                    goff = off
                    gend = int(offs[gstart[g + 1]])
                    at = apool.tile([P, GPW * CBmax * D], f16, tag="attr")
                    nc.sync.dma_start(
                        out=at[:, : (gend - goff) * D],
                        in_=attrT_d[:, goff * D : gend * D],
                    )
                    if j == 0:
                        nc.sync.dma_start(out=c16[:], in_=c16_d[:, :])
                        nc.sync.dma_start(out=cs[:], in_=consts_d[:, :])
                        nc.sync.dma_start(out=ws[:], in_=wts_d[:, :])
                    while pending_out:
                        pn0, pNW, pos_ = pending_out.pop(0)
                        nc.sync.dma_start(
                            out=outT_d[:, pn0 : pn0 + pNW], in_=pos_[:, :pNW]
                        )
                woff = off - goff  # window's chunk offset within group tile

                oh = ohpool.tile([P, CBmax * W], f16, tag="oh")
                if "oh" not in ablate:
                    nc.vector.tensor_tensor(
                        out=oh[:, : cb * W].rearrange("p (c m) -> p c m", m=W),
                        in0=dstrel_s[:, off : off + cb].to_broadcast([P, cb, W]),
                        in1=it16[:, : cb * W].rearrange("p (c m) -> p c m", m=W),
                        op=mybir.AluOpType.is_equal,
                    )

                pm = pbin.tile([P, W], f32, tag="mean")
                for ch in range(cb if "bin" not in ablate else 0):
                    nc.tensor.matmul(
                        out=pm[:],
                        lhsT=at[:, (woff + ch) * D : (woff + ch + 1) * D],
                        rhs=oh[:, ch * W : (ch + 1) * W],
                        start=(ch == 0),
                        stop=(ch == cb - 1),
                    )

                if sw == 0:
                    mean_g = actpool.tile([P, GROUP_N], f32r, tag="mean_g")
                if "bin" not in ablate:
                    nc.scalar.copy(out=mean_g[:, sw * W : (sw + 1) * W], in_=pm[:])

                if ("mlp" not in ablate) and (sw == gsz - 1):
                    # --- MLP over this group of nodes (feature-major) ---
                    NW = gsz * W
                    n0 = gstart[g] * W
                    xg16 = actpool.tile([P, GROUP_N], f16, tag="xg16")
                    nc.sync.dma_start(out=xg16[:, :NW], in_=xT_d[:, n0 : n0 + NW])
                    xg = actpool.tile([P, GROUP_N], f32r, tag="xg")
                    nc.vector.tensor_copy(out=xg[:, :NW], in_=xg16[:, :NW])

                    ph1a = pmlp.tile([P, GROUP_N], f32, tag="h1a")
                    ph1b = pmlp.tile([P, GROUP_N], f32, tag="h1b")
                    nc.tensor.matmul(out=ph1a[:, :NW], lhsT=w1s_0[:, 0:P],
                                     rhs=xg[:, :NW], start=True, stop=False)
                    nc.tensor.matmul(out=ph1a[:, :NW], lhsT=w1s_1[:, 0:P],
                                     rhs=mean_g[:, :NW], start=False, stop=True)
                    nc.tensor.matmul(out=ph1b[:, :NW], lhsT=w1s_0[:, P:HIDDEN],
                                     rhs=xg[:, :NW], start=True, stop=False)
                    nc.tensor.matmul(out=ph1b[:, :NW], lhsT=w1s_1[:, P:HIDDEN],
                                     rhs=mean_g[:, :NW], start=False, stop=True)
                    h1a = actpool.tile([P, GROUP_N], f32r, tag="h1a_s")
                    h1b = actpool.tile([P, GROUP_N], f32r, tag="h1b_s")
                    nc.scalar.activation(out=h1a[:, :NW], in_=ph1a[:, :NW],
                                         func=Relu, bias=b1s_0[:, 0:1])
                    nc.scalar.activation(out=h1b[:, :NW], in_=ph1b[:, :NW],
                                         func=Relu, bias=b1s_1[:, 0:1])

                    ph2a = pmlp.tile([P, GROUP_N], f32, tag="h2a")
                    ph2b = pmlp.tile([P, GROUP_N], f32, tag="h2b")
                    nc.tensor.matmul(out=ph2a[:, :NW], lhsT=w2s_0[:, 0:P],
                                     rhs=h1a[:, :NW], start=True, stop=False)
                    nc.tensor.matmul(out=ph2a[:, :NW], lhsT=w2s_1[:, 0:P],
                                     rhs=h1b[:, :NW], start=False, stop=True)
                    nc.tensor.matmul(out=ph2b[:, :NW], lhsT=w2s_0[:, P:HIDDEN],
                                     rhs=h1a[:, :NW], start=True, stop=False)
                    nc.tensor.matmul(out=ph2b[:, :NW], lhsT=w2s_1[:, P:HIDDEN],
                                     rhs=h1b[:, :NW], start=False, stop=True)
                    h2a = actpool.tile([P, GROUP_N], f32r, tag="h2a_s")
                    h2b = actpool.tile([P, GROUP_N], f32r, tag="h2b_s")
                    nc.scalar.activation(out=h2a[:, :NW], in_=ph2a[:, :NW],
                                         func=Relu, bias=b2s_0[:, 0:1])
                    nc.scalar.activation(out=h2b[:, :NW], in_=ph2b[:, :NW],
                                         func=Relu, bias=b2s_1[:, 0:1])

                    po = pmlp.tile([P, GROUP_N], f32, tag="o")
                    nc.tensor.matmul(out=po[:, :NW], lhsT=w3s_0[:],
                                     rhs=h2a[:, :NW], start=True, stop=False)
                    nc.tensor.matmul(out=po[:, :NW], lhsT=w3s_1[:],
                                     rhs=h2b[:, :NW], start=False, stop=True)
                    os_ = actpool.tile([P, GROUP_N], f16, tag="out_s")
                    nc.scalar.activation(out=os_[:, :NW], in_=po[:, :NW],
                                         func=Ident, bias=b3s[:, 0:1])
                    # defer the store one group so SP issues the next
                    # group's prefetches before blocking on this os_ tile
                    pending_out.append((n0, NW, os_))

            while pending_out:
                pn0, pNW, pos_ = pending_out.pop(0)
                nc.sync.dma_start(out=outT_d[:, pn0 : pn0 + pNW], in_=pos_[:, :pNW])

    # run_bass_via_pjrt (axon path) does not finalize; Bacc needs
    # finalize() to run its compile passes (reg alloc, wait legalization).
    nc.finalize()
    return nc


def _host_prep(x, edge_index, edge_attr):
    """Sort/scale/pad edges; returns (CB, per-core input arrays)."""
    col = np.asarray(edge_index)[1].astype(np.int64)
    x = np.asarray(x, dtype=np.float32)
    counts = np.bincount(col, minlength=N_NODES)
    scale = (1.0 / np.maximum(counts, 1)).astype(np.float32)

    order = np.argsort(col, kind="stable")
    col_s = col[order]
    attr_s = np.asarray(edge_attr, dtype=np.float32)[order]
    attr_s = attr_s * scale[col_s][:, None]

    # per-core, per-window edge counts
    starts = np.empty((N_CORES, WINDOWS + 1), dtype=np.int64)
    for c in range(N_CORES):
        bounds = np.minimum(
            c * NPC_REAL + np.arange(WINDOWS + 1) * W, (c + 1) * NPC_REAL
        )
        starts[c] = np.searchsorted(col_s, bounds)
    cnt = np.diff(starts, axis=1)  # [N_CORES, WINDOWS]

    # Each core processes its windows sorted by descending edge count.
    # Window slot j then holds every core's j-th order statistic, so the
    # cross-core max (CB must be shared, the program is SPMD) wastes far
    # less padding than positional assignment.  Small windows land last,
    # which also shortens the pipeline drain.  Host un-permutes outputs.
    order = np.argsort(-cnt, axis=1, kind="stable")  # [N_CORES, WINDOWS]
    cnt_s = np.take_along_axis(cnt, order, axis=1)

    CB = np.maximum(1, (-(-cnt_s // P)).max(axis=0)).astype(int)  # ceil, >=1
    offs = np.concatenate([[0], np.cumsum(CB)]).astype(np.int64)
    NCH = int(offs[-1])
    E_pad = NCH * P

    per_core = []
    for c in range(N_CORES):
        ordc = order[c]
        cnts = cnt_s[c]                      # counts in processing order
        total = int(cnts.sum())
        # edge source rows (into col_s/attr_s), in processing order
        src_idx = np.concatenate(
            [np.arange(starts[c, w], starts[c, w + 1]) for w in ordc]
        )
        base = np.repeat(offs[:-1] * P, cnts)
        within = np.arange(total) - np.repeat(np.cumsum(cnts) - cnts, cnts)
        edest = base + within

        attr_pad = np.zeros((E_pad, D), np.float32)
        attr_pad[edest] = attr_s[src_idx]
        attrT = np.ascontiguousarray(
            attr_pad.reshape(NCH, P, D)
            .transpose(1, 0, 2)
            .reshape(P, NCH * D)
            .astype(np.float16)
        )

        # dst relative to the processed window's node base
        win_base_proc = c * NPC_REAL + ordc * W  # global node base per slot
        dstrel = np.full((E_pad,), 200.0, np.float16)
        dstrel[edest] = (
            col_s[src_idx] - np.repeat(win_base_proc, cnts)
        ).astype(np.float16)
        dstrelT = np.ascontiguousarray(dstrel.reshape(NCH, P).T)

        # node features per 64-node window slot, zero-padded per slot
        xc = np.zeros((WINDOWS, W, D), np.float16)
        for j, w in enumerate(ordc):
            n0 = c * NPC_REAL + w * W
            n1 = min(n0 + W, (c + 1) * NPC_REAL)
            xc[j, : n1 - n0] = x[n0:n1].astype(np.float16)
        xT = np.ascontiguousarray(xc.reshape(NPC, D).T)

        per_core.append(
            {"xT": xT, "attrT": attrT, "dstrelT": dstrelT, "order": ordc}
        )
    return tuple(CB.tolist()), per_core


def _build_consts(b1, b2, b3):
    consts = np.zeros((P, 5), np.float32)
    consts[:, 0] = b1[:P]
    consts[:, 1] = b1[P:]
    consts[:, 2] = b2[:P]
    consts[:, 3] = b2[P:]
    consts[:, 4] = b3
    return consts


def _build_wts(W1, W2, W3):
    wts = np.empty((P, 4 * HIDDEN + 2 * DOUT), np.float32)
    wts[:, 0:HIDDEN] = W1[:P]
    wts[:, HIDDEN : 2 * HIDDEN] = W1[P:]
    wts[:, 2 * HIDDEN : 3 * HIDDEN] = W2[:P]
    wts[:, 3 * HIDDEN : 4 * HIDDEN] = W2[P:]
    wts[:, 4 * HIDDEN : 4 * HIDDEN + DOUT] = W3[:P]
    wts[:, 4 * HIDDEN + DOUT : 4 * HIDDEN + 2 * DOUT] = W3[P:]
    return wts


def _build_c16(CB, dstrelT):
    """fp16 consts row-block: iota ramp | dstrel."""
    CBmax = max(CB)
    NCH = int(sum(CB))
    c16 = np.empty((P, CBmax * W + NCH), np.float16)
    c16[:, 0 : CBmax * W] = np.tile(np.arange(W, dtype=np.float16), CBmax)[None, :]
    c16[:, CBmax * W :] = dstrelT
    return c16


def kernel(x, edge_index, edge_attr, W1, b1, W2, b2, W3, b3):
    CB, per_core = _host_prep(x, edge_index, edge_attr)

    key = CB
    if key not in _prog_cache:
        _prog_cache[key] = _build_program(CB)
    nc = _prog_cache[key]

    W1 = np.asarray(W1, np.float32)
    W2 = np.asarray(W2, np.float32)
    W3 = np.asarray(W3, np.float32)
    b1 = np.asarray(b1, np.float32)
    b2 = np.asarray(b2, np.float32)
    b3 = np.asarray(b3, np.float32)
    consts = _build_consts(b1, b2, b3)
    wts = _build_wts(W1, W2, W3)
    in_maps = [
        {
            "xT": pc["xT"],
            "attrT": pc["attrT"],
            "c16": _build_c16(CB, pc["dstrelT"]),
            "consts": consts,
            "wts": wts,
        }
        for pc in per_core
    ]

    res = run_bass_kernel_spmd(nc, in_maps, core_ids=list(range(N_CORES)))

    out = np.empty((N_NODES, DOUT), np.float32)
    for c in range(N_CORES):
        o = res.results[c]["outT"].T.astype(np.float32).reshape(WINDOWS, W, DOUT)
        for j, w in enumerate(per_core[c]["order"]):
            n0 = c * NPC_REAL + int(w) * W
            n1 = min(n0 + W, (c + 1) * NPC_REAL)
            out[n0:n1] = o[j, : n1 - n0]
    return out


# revision 38
# speedup vs baseline: 1.2723x; 1.0034x over previous
"""NodeNet GNN message-passing kernel for 8 Trainium2 NeuronCores.

Strategy (per sharding hint): shard nodes across the 8 cores; partition
edges by destination node on the host so the scatter-mean is device-local.

Per core (12,500 real nodes, padded to 12,544 = 196 windows of 64 nodes):
  - Host sorts edges by destination, pre-scales each edge row by
    1/count(dst) (so the segment-sum directly yields the mean), pads each
    64-node window's edge list to a multiple of 128, and lays edge
    features out chunk-transposed ([128, nch*128] fp16) so all device
    DMAs are wide and contiguous.
  - Device builds, per 128-edge chunk, a [128 edge, 64 node] fp16 one-hot
    (is_equal of dst-rel against an iota ramp) and uses the TensorEngine:
    meanT[d, n] += matmul(lhsT=attr[e,d], rhs=onehot[e,n]) accumulated in
    PSUM (fp32).  Everything stays feature-major so the 3-layer MLP
    (float32r matmuls) chains without transposes:
    h1T = relu(W1.T @ [xT; meanT] + b1), etc.
  - Output is written feature-major fp16 [128, 12544]; host transposes
    and upcasts.
"""

import numpy as np

import concourse.bacc as bacc
import concourse.mybir as mybir
import concourse.tile as tile
from concourse.bass_utils import run_bass_kernel_spmd

P = 128                    # partitions / matmul contraction tile
D = 128                    # node & edge feature dim
HIDDEN = 256
DOUT = 128
N_NODES = 100000
N_CORES = 8
NPC_REAL = 12500           # real nodes per core
W = 64                     # nodes per binning window
WINDOWS = 196              # windows per core (196*64 = 12544)
NPC = WINDOWS * W          # padded nodes per core
GPW = 8                    # windows per MLP group (512 nodes)
GROUP_N = GPW * W
ATTR_BUFS = 3
OH_BUFS = 4
ACT_BUFS = 3
PBIN_BUFS = 3

_prog_cache: dict = {}

f32 = mybir.dt.float32
f16 = mybir.dt.float16
f32r = mybir.dt.float32r


def _group_sizes():
    gsizes = []
    rem = WINDOWS
    while rem > 2 * GPW:
        gsizes.append(GPW)
        rem -= GPW
    for t in (GPW // 2, GPW // 2, GPW // 4, GPW // 4):
        t = min(t, rem)
        if t > 0:
            gsizes.append(t)
            rem -= t
    while rem > 0:
        t = min(GPW // 4, rem)
        gsizes.append(t)
        rem -= t
    return gsizes


def _build_program(CB, ablate=()):
    """Build the Bass/Tile program. CB[j] = number of 128-edge chunks for
    window j (identical across cores; per-core data is padded to match).
    ablate: subset of {"mlp", "bin", "oh"} to skip stages (sim studies)."""
    CB = list(CB)
    CBmax = max(CB)
    offs = np.concatenate([[0], np.cumsum(CB)]).astype(int)
    NCH = int(offs[-1])

    nc = bacc.Bacc(None)
    # attrT carries, per group: the edge-feature chunks, then the group's
    # node features (gsz*W fp16 columns) — one combined DMA per group.
    attrT_d = nc.dram_tensor(
        "attrT", [P, NCH * D + WINDOWS * W], f16, kind="ExternalInput"
    )
    # fp16 consts: iota ramp (CBmax*W) | dstrel (NCH)
    c16_d = nc.dram_tensor("c16", [P, CBmax * W + NCH], f16, kind="ExternalInput")
    # fp32 consts: 5 bias columns
    consts_d = nc.dram_tensor("consts", [P, 5], f32, kind="ExternalInput")
    wts_d = nc.dram_tensor("wts", [P, 4 * HIDDEN + 2 * DOUT], f32r,
                           kind="ExternalInput")
    outT_d = nc.dram_tensor("outT", [P, NPC], f16, kind="ExternalOutput")

    Relu = mybir.ActivationFunctionType.Relu
    Ident = mybir.ActivationFunctionType.Identity

    with tile.TileContext(nc) as tc:
        with (
            tc.tile_pool(name="const", bufs=1) as cpool,
            tc.tile_pool(name="attr", bufs=ATTR_BUFS) as apool,
            tc.tile_pool(name="oh", bufs=OH_BUFS) as ohpool,
            tc.tile_pool(name="acts", bufs=ACT_BUFS) as actpool,
            tc.tile_pool(name="pbin", bufs=PBIN_BUFS, space="PSUM") as pbin,
            tc.tile_pool(name="pmlp", bufs=1, space="PSUM") as pmlp,
        ):
            # --- constants (tiles now; DMAs after the first attr DMA so
            # the edge stream starts immediately) ---
            cs = cpool.tile([P, 5], f32, tag="consts")
            ws = cpool.tile([P, 4 * HIDDEN + 2 * DOUT], f32r, tag="wts")
            c16 = cpool.tile([P, CBmax * W + NCH], f16, tag="c16")
            w1s_0 = ws[:, 0:HIDDEN]
            w1s_1 = ws[:, HIDDEN : 2 * HIDDEN]
            w2s_0 = ws[:, 2 * HIDDEN : 3 * HIDDEN]
            w2s_1 = ws[:, 3 * HIDDEN : 4 * HIDDEN]
            w3s_0 = ws[:, 4 * HIDDEN : 4 * HIDDEN + DOUT]
            w3s_1 = ws[:, 4 * HIDDEN + DOUT : 4 * HIDDEN + 2 * DOUT]
            b1s_0 = cs[:, 0:1]
            b1s_1 = cs[:, 1:2]
            b2s_0 = cs[:, 2:3]
            b2s_1 = cs[:, 3:4]
            b3s = cs[:, 4:5]
            it16 = c16[:, 0 : CBmax * W]
            dstrel_s = c16[:, CBmax * W : CBmax * W + NCH]
            oall = cpool.tile([P, NPC], f16, tag="oall")

            # group sizes: GPW windows each, tapering at the tail to
            # shorten the pipeline drain (last windows are also the
            # smallest thanks to the descending-count permutation)
            gsizes = _group_sizes()
            gstart = [0]
            for s in gsizes:
                gstart.append(gstart[-1] + s)

            for j in range(WINDOWS):
                cb = CB[j]
                off = int(offs[j])
                g = next(i for i in range(len(gsizes)) if gstart[i + 1] > j)
                sw = j - gstart[g]
                gsz = gsizes[g]

                if sw == 0:
                    # one combined edge-feature + node-feature DMA per group
                    goff = off
                    gend = int(offs[gstart[g + 1]])
                    gw = (gend - goff) * D + gsz * W
                    gsrc = goff * D + gstart[g] * W
                    at = apool.tile([P, GPW * (CBmax * D + W)], f16, tag="attr")
                    nc.sync.dma_start(
                        out=at[:, :gw], in_=attrT_d[:, gsrc : gsrc + gw]
                    )
                    if j == 0:
                        nc.sync.dma_start(out=c16[:], in_=c16_d[:, :])
                        nc.sync.dma_start(out=cs[:], in_=consts_d[:, :])
                        nc.sync.dma_start(out=ws[:], in_=wts_d[:, :])
                    # flush the previous group's finished output slice
                    if g > 0:
                        f0, f1 = gstart[g - 1] * W, gstart[g] * W
                        nc.sync.dma_start(
                            out=outT_d[:, f0:f1], in_=oall[:, f0:f1]
                        )
                woff = off - goff  # window's chunk offset within group tile

                oh = ohpool.tile([P, CBmax * W], f16, tag="oh")
                if "oh" not in ablate:
                    nc.vector.tensor_tensor(
                        out=oh[:, : cb * W].rearrange("p (c m) -> p c m", m=W),
                        in0=dstrel_s[:, off : off + cb].to_broadcast([P, cb, W]),
                        in1=it16[:, : cb * W].rearrange("p (c m) -> p c m", m=W),
                        op=mybir.AluOpType.is_equal,
                    )

                pm = pbin.tile([P, W], f32, tag="mean")
                for ch in range(cb if "bin" not in ablate else 0):
                    nc.tensor.matmul(
                        out=pm[:],
                        lhsT=at[:, (woff + ch) * D : (woff + ch + 1) * D],
                        rhs=oh[:, ch * W : (ch + 1) * W],
                        start=(ch == 0),
                        stop=(ch == cb - 1),
                    )

                if sw == 0:
                    mean_g = actpool.tile([P, GROUP_N], f32r, tag="mean_g")
                if "bin" not in ablate:
                    nc.scalar.copy(out=mean_g[:, sw * W : (sw + 1) * W], in_=pm[:])

                if ("mlp" not in ablate) and (sw == gsz - 1):
                    # --- MLP over this group of nodes (feature-major) ---
                    NW = gsz * W
                    n0 = gstart[g] * W
                    xg = actpool.tile([P, GROUP_N], f32r, tag="xg")
                    nc.vector.tensor_copy(
                        out=xg[:, :NW],
                        in_=at[:, (gend - goff) * D : (gend - goff) * D + NW],
                    )

                    ph1a = pmlp.tile([P, GROUP_N], f32, tag="h1a")
                    ph1b = pmlp.tile([P, GROUP_N], f32, tag="h1b")
                    nc.tensor.matmul(out=ph1a[:, :NW], lhsT=w1s_0[:, 0:P],
                                     rhs=xg[:, :NW], start=True, stop=False)
                    nc.tensor.matmul(out=ph1a[:, :NW], lhsT=w1s_1[:, 0:P],
                                     rhs=mean_g[:, :NW], start=False, stop=True)
                    nc.tensor.matmul(out=ph1b[:, :NW], lhsT=w1s_0[:, P:HIDDEN],
                                     rhs=xg[:, :NW], start=True, stop=False)
                    nc.tensor.matmul(out=ph1b[:, :NW], lhsT=w1s_1[:, P:HIDDEN],
                                     rhs=mean_g[:, :NW], start=False, stop=True)
                    h1a = actpool.tile([P, GROUP_N], f32r, tag="h1a_s")
                    h1b = actpool.tile([P, GROUP_N], f32r, tag="h1b_s")
                    nc.scalar.activation(out=h1a[:, :NW], in_=ph1a[:, :NW],
                                         func=Relu, bias=b1s_0[:, 0:1])
                    nc.scalar.activation(out=h1b[:, :NW], in_=ph1b[:, :NW],
                                         func=Relu, bias=b1s_1[:, 0:1])

                    ph2a = pmlp.tile([P, GROUP_N], f32, tag="h2a")
                    ph2b = pmlp.tile([P, GROUP_N], f32, tag="h2b")
                    nc.tensor.matmul(out=ph2a[:, :NW], lhsT=w2s_0[:, 0:P],
                                     rhs=h1a[:, :NW], start=True, stop=False)
                    nc.tensor.matmul(out=ph2a[:, :NW], lhsT=w2s_1[:, 0:P],
                                     rhs=h1b[:, :NW], start=False, stop=True)
                    nc.tensor.matmul(out=ph2b[:, :NW], lhsT=w2s_0[:, P:HIDDEN],
                                     rhs=h1a[:, :NW], start=True, stop=False)
                    nc.tensor.matmul(out=ph2b[:, :NW], lhsT=w2s_1[:, P:HIDDEN],
                                     rhs=h1b[:, :NW], start=False, stop=True)
                    h2a = actpool.tile([P, GROUP_N], f32r, tag="h2a_s")
                    h2b = actpool.tile([P, GROUP_N], f32r, tag="h2b_s")
                    nc.scalar.activation(out=h2a[:, :NW], in_=ph2a[:, :NW],
                                         func=Relu, bias=b2s_0[:, 0:1])
                    nc.scalar.activation(out=h2b[:, :NW], in_=ph2b[:, :NW],
                                         func=Relu, bias=b2s_1[:, 0:1])

                    po = pmlp.tile([P, GROUP_N], f32, tag="o")
                    nc.tensor.matmul(out=po[:, :NW], lhsT=w3s_0[:],
                                     rhs=h2a[:, :NW], start=True, stop=False)
                    nc.tensor.matmul(out=po[:, :NW], lhsT=w3s_1[:],
                                     rhs=h2b[:, :NW], start=False, stop=True)
                    nc.scalar.activation(out=oall[:, n0 : n0 + NW],
                                         in_=po[:, :NW],
                                         func=Ident, bias=b3s[:, 0:1])

            f0 = gstart[len(gsizes) - 1] * W
            nc.sync.dma_start(out=outT_d[:, f0:], in_=oall[:, f0:])

    # run_bass_via_pjrt (axon path) does not finalize; Bacc needs
    # finalize() to run its compile passes (reg alloc, wait legalization).
    nc.finalize()
    return nc


def _host_prep(x, edge_index, edge_attr):
    """Sort/scale/pad edges; returns (CB, per-core input arrays)."""
    col = np.asarray(edge_index)[1].astype(np.int64)
    x = np.asarray(x, dtype=np.float32)
    counts = np.bincount(col, minlength=N_NODES)
    scale = (1.0 / np.maximum(counts, 1)).astype(np.float32)

    order = np.argsort(col, kind="stable")
    col_s = col[order]
    attr_s = np.asarray(edge_attr, dtype=np.float32)[order]
    attr_s = attr_s * scale[col_s][:, None]

    # per-core, per-window edge counts
    starts = np.empty((N_CORES, WINDOWS + 1), dtype=np.int64)
    for c in range(N_CORES):
        bounds = np.minimum(
            c * NPC_REAL + np.arange(WINDOWS + 1) * W, (c + 1) * NPC_REAL
        )
        starts[c] = np.searchsorted(col_s, bounds)
    cnt = np.diff(starts, axis=1)  # [N_CORES, WINDOWS]

    # Each core processes its windows sorted by descending edge count.
    # Window slot j then holds every core's j-th order statistic, so the
    # cross-core max (CB must be shared, the program is SPMD) wastes far
    # less padding than positional assignment.  Small windows land last,
    # which also shortens the pipeline drain.  Host un-permutes outputs.
    order = np.argsort(-cnt, axis=1, kind="stable")  # [N_CORES, WINDOWS]
    cnt_s = np.take_along_axis(cnt, order, axis=1)

    CB = np.maximum(1, (-(-cnt_s // P)).max(axis=0)).astype(int)  # ceil, >=1
    offs = np.concatenate([[0], np.cumsum(CB)]).astype(np.int64)
    NCH = int(offs[-1])
    E_pad = NCH * P

    per_core = []
    for c in range(N_CORES):
        ordc = order[c]
        cnts = cnt_s[c]                      # counts in processing order
        total = int(cnts.sum())
        # edge source rows (into col_s/attr_s), in processing order
        src_idx = np.concatenate(
            [np.arange(starts[c, w], starts[c, w + 1]) for w in ordc]
        )
        base = np.repeat(offs[:-1] * P, cnts)
        within = np.arange(total) - np.repeat(np.cumsum(cnts) - cnts, cnts)
        edest = base + within

        attr_pad = np.zeros((E_pad, D), np.float32)
        attr_pad[edest] = attr_s[src_idx]
        attrT_edges = (
            attr_pad.reshape(NCH, P, D)
            .transpose(1, 0, 2)
            .reshape(P, NCH * D)
            .astype(np.float16)
        )

        # dst relative to the processed window's node base
        win_base_proc = c * NPC_REAL + ordc * W  # global node base per slot
        dstrel = np.full((E_pad,), 200.0, np.float16)
        dstrel[edest] = (
            col_s[src_idx] - np.repeat(win_base_proc, cnts)
        ).astype(np.float16)
        dstrelT = np.ascontiguousarray(dstrel.reshape(NCH, P).T)

        # node features per 64-node window slot, zero-padded per slot
        xc = np.zeros((WINDOWS, W, D), np.float16)
        for j, w in enumerate(ordc):
            n0 = c * NPC_REAL + w * W
            n1 = min(n0 + W, (c + 1) * NPC_REAL)
            xc[j, : n1 - n0] = x[n0:n1].astype(np.float16)
        xT = xc.reshape(NPC, D).T  # [D, NPC]

        # interleave per group: [edge chunks | node features]
        gsizes = _group_sizes()
        attrT = np.empty((P, NCH * D + WINDOWS * W), np.float16)
        pos = 0
        j0 = 0
        for gsz in gsizes:
            c0, c1 = int(offs[j0]), int(offs[j0 + gsz])
            wgt = (c1 - c0) * D
            attrT[:, pos : pos + wgt] = attrT_edges[:, c0 * D : c1 * D]
            pos += wgt
            attrT[:, pos : pos + gsz * W] = xT[:, j0 * W : (j0 + gsz) * W]
            pos += gsz * W
            j0 += gsz
        assert pos == attrT.shape[1] and j0 == WINDOWS

        per_core.append(
            {"attrT": np.ascontiguousarray(attrT), "dstrelT": dstrelT,
             "order": ordc}
        )
    return tuple(CB.tolist()), per_core


def _build_consts(b1, b2, b3):
    consts = np.zeros((P, 5), np.float32)
    consts[:, 0] = b1[:P]
    consts[:, 1] = b1[P:]
    consts[:, 2] = b2[:P]
    consts[:, 3] = b2[P:]
    consts[:, 4] = b3
    return consts


def _build_wts(W1, W2, W3):
    wts = np.empty((P, 4 * HIDDEN + 2 * DOUT), np.float32)
    wts[:, 0:HIDDEN] = W1[:P]
    wts[:, HIDDEN : 2 * HIDDEN] = W1[P:]
    wts[:, 2 * HIDDEN : 3 * HIDDEN] = W2[:P]
    wts[:, 3 * HIDDEN : 4 * HIDDEN] = W2[P:]
    wts[:, 4 * HIDDEN : 4 * HIDDEN + DOUT] = W3[:P]
    wts[:, 4 * HIDDEN + DOUT : 4 * HIDDEN + 2 * DOUT] = W3[P:]
    return wts


def _build_c16(CB, dstrelT):
    """fp16 consts row-block: iota ramp | dstrel."""
    CBmax = max(CB)
    NCH = int(sum(CB))
    c16 = np.empty((P, CBmax * W + NCH), np.float16)
    c16[:, 0 : CBmax * W] = np.tile(np.arange(W, dtype=np.float16), CBmax)[None, :]
    c16[:, CBmax * W :] = dstrelT
    return c16


def kernel(x, edge_index, edge_attr, W1, b1, W2, b2, W3, b3):
    CB, per_core = _host_prep(x, edge_index, edge_attr)

    key = CB
    if key not in _prog_cache:
        _prog_cache[key] = _build_program(CB)
    nc = _prog_cache[key]

    W1 = np.asarray(W1, np.float32)
    W2 = np.asarray(W2, np.float32)
    W3 = np.asarray(W3, np.float32)
    b1 = np.asarray(b1, np.float32)
    b2 = np.asarray(b2, np.float32)
    b3 = np.asarray(b3, np.float32)
    consts = _build_consts(b1, b2, b3)
    wts = _build_wts(W1, W2, W3)
    in_maps = [
        {
            "attrT": pc["attrT"],
            "c16": _build_c16(CB, pc["dstrelT"]),
            "consts": consts,
            "wts": wts,
        }
        for pc in per_core
    ]

    res = run_bass_kernel_spmd(nc, in_maps, core_ids=list(range(N_CORES)))

    out = np.empty((N_NODES, DOUT), np.float32)
    for c in range(N_CORES):
        o = res.results[c]["outT"].T.astype(np.float32).reshape(WINDOWS, W, DOUT)
        for j, w in enumerate(per_core[c]["order"]):
            n0 = c * NPC_REAL + int(w) * W
            n1 = min(n0 + W, (c + 1) * NPC_REAL)
            out[n0:n1] = o[j, : n1 - n0]
    return out


# revision 39
# speedup vs baseline: 1.2769x; 1.0036x over previous
"""NodeNet GNN message-passing kernel for 8 Trainium2 NeuronCores.

Strategy (per sharding hint): shard nodes across the 8 cores; partition
edges by destination node on the host so the scatter-mean is device-local.

Per core (12,500 real nodes, padded to 12,544 = 196 windows of 64 nodes):
  - Host sorts edges by destination, pre-scales each edge row by
    1/count(dst) (so the segment-sum directly yields the mean), pads each
    64-node window's edge list to a multiple of 128, and lays edge
    features out chunk-transposed ([128, nch*128] fp16) so all device
    DMAs are wide and contiguous.
  - Device builds, per 128-edge chunk, a [128 edge, 64 node] fp16 one-hot
    (is_equal of dst-rel against an iota ramp) and uses the TensorEngine:
    meanT[d, n] += matmul(lhsT=attr[e,d], rhs=onehot[e,n]) accumulated in
    PSUM (fp32).  Everything stays feature-major so the 3-layer MLP
    (float32r matmuls) chains without transposes:
    h1T = relu(W1.T @ [xT; meanT] + b1), etc.
  - Output is written feature-major fp16 [128, 12544]; host transposes
    and upcasts.
"""

import numpy as np

import concourse.bacc as bacc
import concourse.mybir as mybir
import concourse.tile as tile
from concourse.bass_utils import run_bass_kernel_spmd

P = 128                    # partitions / matmul contraction tile
D = 128                    # node & edge feature dim
HIDDEN = 256
DOUT = 128
N_NODES = 100000
N_CORES = 8
NPC_REAL = 12500           # real nodes per core
W = 64                     # nodes per binning window
WINDOWS = 196              # windows per core (196*64 = 12544)
NPC = WINDOWS * W          # padded nodes per core
GPW = 8                    # windows per MLP group (512 nodes)
GROUP_N = GPW * W
ATTR_BUFS = 3
OH_BUFS = 4
ACT_BUFS = 3
PBIN_BUFS = 3

_prog_cache: dict = {}

f32 = mybir.dt.float32
f16 = mybir.dt.float16
f32r = mybir.dt.float32r


def _group_sizes():
    gsizes = []
    rem = WINDOWS
    while rem > 2 * GPW:
        gsizes.append(GPW)
        rem -= GPW
    for t in (GPW // 2, GPW // 2, GPW // 4, GPW // 4):
        t = min(t, rem)
        if t > 0:
            gsizes.append(t)
            rem -= t
    while rem > 0:
        t = min(GPW // 4, rem)
        gsizes.append(t)
        rem -= t
    return gsizes


def _build_program(CB, ablate=()):
    """Build the Bass/Tile program. CB[j] = number of 128-edge chunks for
    window j (identical across cores; per-core data is padded to match).
    ablate: subset of {"mlp", "bin", "oh"} to skip stages (sim studies)."""
    CB = list(CB)
    CBmax = max(CB)
    offs = np.concatenate([[0], np.cumsum(CB)]).astype(int)
    NCH = int(offs[-1])

    nc = bacc.Bacc(None)
    # attrT carries, per group: the edge-feature chunks, then the group's
    # node features (gsz*W fp16 columns) — one combined DMA per group.
    attrT_d = nc.dram_tensor(
        "attrT", [P, NCH * D + WINDOWS * W], f16, kind="ExternalInput"
    )
    # fp16 consts: iota ramp (CBmax*W) | dstrel (NCH)
    c16_d = nc.dram_tensor("c16", [P, CBmax * W + NCH], f16, kind="ExternalInput")
    # fp32 consts: 5 bias columns
    consts_d = nc.dram_tensor("consts", [P, 5], f32, kind="ExternalInput")
    wts_d = nc.dram_tensor("wts", [P, 4 * HIDDEN + 2 * DOUT], f16,
                           kind="ExternalInput")
    outT_d = nc.dram_tensor("outT", [P, NPC], f16, kind="ExternalOutput")

    Relu = mybir.ActivationFunctionType.Relu
    Ident = mybir.ActivationFunctionType.Identity

    with tile.TileContext(nc) as tc:
        with (
            tc.tile_pool(name="const", bufs=1) as cpool,
            tc.tile_pool(name="attr", bufs=ATTR_BUFS) as apool,
            tc.tile_pool(name="oh", bufs=OH_BUFS) as ohpool,
            tc.tile_pool(name="acts", bufs=ACT_BUFS) as actpool,
            tc.tile_pool(name="pbin", bufs=PBIN_BUFS, space="PSUM") as pbin,
            tc.tile_pool(name="pmlp", bufs=1, space="PSUM") as pmlp,
        ):
            # --- constants (tiles now; DMAs after the first attr DMA so
            # the edge stream starts immediately) ---
            cs = cpool.tile([P, 5], f32, tag="consts")
            ws = cpool.tile([P, 4 * HIDDEN + 2 * DOUT], f16, tag="wts")
            c16 = cpool.tile([P, CBmax * W + NCH], f16, tag="c16")
            w1s_0 = ws[:, 0:HIDDEN]
            w1s_1 = ws[:, HIDDEN : 2 * HIDDEN]
            w2s_0 = ws[:, 2 * HIDDEN : 3 * HIDDEN]
            w2s_1 = ws[:, 3 * HIDDEN : 4 * HIDDEN]
            w3s_0 = ws[:, 4 * HIDDEN : 4 * HIDDEN + DOUT]
            w3s_1 = ws[:, 4 * HIDDEN + DOUT : 4 * HIDDEN + 2 * DOUT]
            b1s_0 = cs[:, 0:1]
            b1s_1 = cs[:, 1:2]
            b2s_0 = cs[:, 2:3]
            b2s_1 = cs[:, 3:4]
            b3s = cs[:, 4:5]
            it16 = c16[:, 0 : CBmax * W]
            dstrel_s = c16[:, CBmax * W : CBmax * W + NCH]
            oall = cpool.tile([P, NPC], f16, tag="oall")

            # group sizes: GPW windows each, tapering at the tail to
            # shorten the pipeline drain (last windows are also the
            # smallest thanks to the descending-count permutation)
            gsizes = _group_sizes()
            gstart = [0]
            for s in gsizes:
                gstart.append(gstart[-1] + s)

            for j in range(WINDOWS):
                cb = CB[j]
                off = int(offs[j])
                g = next(i for i in range(len(gsizes)) if gstart[i + 1] > j)
                sw = j - gstart[g]
                gsz = gsizes[g]

                if sw == 0:
                    # one combined edge-feature + node-feature DMA per group
                    goff = off
                    gend = int(offs[gstart[g + 1]])
                    gw = (gend - goff) * D + gsz * W
                    gsrc = goff * D + gstart[g] * W
                    at = apool.tile([P, GPW * (CBmax * D + W)], f16, tag="attr")
                    nc.sync.dma_start(
                        out=at[:, :gw], in_=attrT_d[:, gsrc : gsrc + gw]
                    )
                    if j == 0:
                        nc.sync.dma_start(out=c16[:], in_=c16_d[:, :])
                        nc.sync.dma_start(out=cs[:], in_=consts_d[:, :])
                        nc.sync.dma_start(out=ws[:], in_=wts_d[:, :])
                    # flush the previous group's finished output slice
                    if g > 0:
                        f0, f1 = gstart[g - 1] * W, gstart[g] * W
                        nc.sync.dma_start(
                            out=outT_d[:, f0:f1], in_=oall[:, f0:f1]
                        )
                woff = off - goff  # window's chunk offset within group tile

                oh = ohpool.tile([P, CBmax * W], f16, tag="oh")
                if "oh" not in ablate:
                    nc.vector.tensor_tensor(
                        out=oh[:, : cb * W].rearrange("p (c m) -> p c m", m=W),
                        in0=dstrel_s[:, off : off + cb].to_broadcast([P, cb, W]),
                        in1=it16[:, : cb * W].rearrange("p (c m) -> p c m", m=W),
                        op=mybir.AluOpType.is_equal,
                    )

                pm = pbin.tile([P, W], f32, tag="mean")
                for ch in range(cb if "bin" not in ablate else 0):
                    nc.tensor.matmul(
                        out=pm[:],
                        lhsT=at[:, (woff + ch) * D : (woff + ch + 1) * D],
                        rhs=oh[:, ch * W : (ch + 1) * W],
                        start=(ch == 0),
                        stop=(ch == cb - 1),
                    )

                if sw == 0:
                    mean_g = actpool.tile([P, GROUP_N], f16, tag="mean_g")
                if "bin" not in ablate:
                    nc.scalar.copy(out=mean_g[:, sw * W : (sw + 1) * W], in_=pm[:])

                if ("mlp" not in ablate) and (sw == gsz - 1):
                    # --- MLP over this group of nodes (feature-major) ---
                    NW = gsz * W
                    n0 = gstart[g] * W

                    ph1a = pmlp.tile([P, GROUP_N], f32, tag="h1a")
                    ph1b = pmlp.tile([P, GROUP_N], f32, tag="h1b")
                    nc.tensor.matmul(out=ph1a[:, :NW], lhsT=w1s_0[:, 0:P],
                                     rhs=at[:, (gend - goff) * D : (gend - goff) * D + NW], start=True, stop=False)
                    nc.tensor.matmul(out=ph1a[:, :NW], lhsT=w1s_1[:, 0:P],
                                     rhs=mean_g[:, :NW], start=False, stop=True)
                    nc.tensor.matmul(out=ph1b[:, :NW], lhsT=w1s_0[:, P:HIDDEN],
                                     rhs=at[:, (gend - goff) * D : (gend - goff) * D + NW], start=True, stop=False)
                    nc.tensor.matmul(out=ph1b[:, :NW], lhsT=w1s_1[:, P:HIDDEN],
                                     rhs=mean_g[:, :NW], start=False, stop=True)
                    h1a = actpool.tile([P, GROUP_N], f16, tag="h1a_s")
                    h1b = actpool.tile([P, GROUP_N], f16, tag="h1b_s")
                    nc.scalar.activation(out=h1a[:, :NW], in_=ph1a[:, :NW],
                                         func=Relu, bias=b1s_0[:, 0:1])
                    nc.scalar.activation(out=h1b[:, :NW], in_=ph1b[:, :NW],
                                         func=Relu, bias=b1s_1[:, 0:1])

                    ph2a = pmlp.tile([P, GROUP_N], f32, tag="h2a")
                    ph2b = pmlp.tile([P, GROUP_N], f32, tag="h2b")
                    nc.tensor.matmul(out=ph2a[:, :NW], lhsT=w2s_0[:, 0:P],
                                     rhs=h1a[:, :NW], start=True, stop=False)
                    nc.tensor.matmul(out=ph2a[:, :NW], lhsT=w2s_1[:, 0:P],
                                     rhs=h1b[:, :NW], start=False, stop=True)
                    nc.tensor.matmul(out=ph2b[:, :NW], lhsT=w2s_0[:, P:HIDDEN],
                                     rhs=h1a[:, :NW], start=True, stop=False)
                    nc.tensor.matmul(out=ph2b[:, :NW], lhsT=w2s_1[:, P:HIDDEN],
                                     rhs=h1b[:, :NW], start=False, stop=True)
                    h2a = actpool.tile([P, GROUP_N], f16, tag="h2a_s")
                    h2b = actpool.tile([P, GROUP_N], f16, tag="h2b_s")
                    nc.scalar.activation(out=h2a[:, :NW], in_=ph2a[:, :NW],
                                         func=Relu, bias=b2s_0[:, 0:1])
                    nc.scalar.activation(out=h2b[:, :NW], in_=ph2b[:, :NW],
                                         func=Relu, bias=b2s_1[:, 0:1])

                    po = pmlp.tile([P, GROUP_N], f32, tag="o")
                    nc.tensor.matmul(out=po[:, :NW], lhsT=w3s_0[:],
                                     rhs=h2a[:, :NW], start=True, stop=False)
                    nc.tensor.matmul(out=po[:, :NW], lhsT=w3s_1[:],
                                     rhs=h2b[:, :NW], start=False, stop=True)
                    nc.scalar.activation(out=oall[:, n0 : n0 + NW],
                                         in_=po[:, :NW],
                                         func=Ident, bias=b3s[:, 0:1])

            f0 = gstart[len(gsizes) - 1] * W
            nc.sync.dma_start(out=outT_d[:, f0:], in_=oall[:, f0:])

    # run_bass_via_pjrt (axon path) does not finalize; Bacc needs
    # finalize() to run its compile passes (reg alloc, wait legalization).
    nc.finalize()
    return nc


def _host_prep(x, edge_index, edge_attr):
    """Sort/scale/pad edges; returns (CB, per-core input arrays)."""
    col = np.asarray(edge_index)[1].astype(np.int64)
    x = np.asarray(x, dtype=np.float32)
    counts = np.bincount(col, minlength=N_NODES)
    scale = (1.0 / np.maximum(counts, 1)).astype(np.float32)

    order = np.argsort(col, kind="stable")
    col_s = col[order]
    attr_s = np.asarray(edge_attr, dtype=np.float32)[order]
    attr_s = attr_s * scale[col_s][:, None]

    # per-core, per-window edge counts
    starts = np.empty((N_CORES, WINDOWS + 1), dtype=np.int64)
    for c in range(N_CORES):
        bounds = np.minimum(
            c * NPC_REAL + np.arange(WINDOWS + 1) * W, (c + 1) * NPC_REAL
        )
        starts[c] = np.searchsorted(col_s, bounds)
    cnt = np.diff(starts, axis=1)  # [N_CORES, WINDOWS]

    # Each core processes its windows sorted by descending edge count.
    # Window slot j then holds every core's j-th order statistic, so the
    # cross-core max (CB must be shared, the program is SPMD) wastes far
    # less padding than positional assignment.  Small windows land last,
    # which also shortens the pipeline drain.  Host un-permutes outputs.
    order = np.argsort(-cnt, axis=1, kind="stable")  # [N_CORES, WINDOWS]
    cnt_s = np.take_along_axis(cnt, order, axis=1)

    CB = np.maximum(1, (-(-cnt_s // P)).max(axis=0)).astype(int)  # ceil, >=1
    offs = np.concatenate([[0], np.cumsum(CB)]).astype(np.int64)
    NCH = int(offs[-1])
    E_pad = NCH * P

    per_core = []
    for c in range(N_CORES):
        ordc = order[c]
        cnts = cnt_s[c]                      # counts in processing order
        total = int(cnts.sum())
        # edge source rows (into col_s/attr_s), in processing order
        src_idx = np.concatenate(
            [np.arange(starts[c, w], starts[c, w + 1]) for w in ordc]
        )
        base = np.repeat(offs[:-1] * P, cnts)
        within = np.arange(total) - np.repeat(np.cumsum(cnts) - cnts, cnts)
        edest = base + within

        attr_pad = np.zeros((E_pad, D), np.float32)
        attr_pad[edest] = attr_s[src_idx]
        attrT_edges = (
            attr_pad.reshape(NCH, P, D)
            .transpose(1, 0, 2)
            .reshape(P, NCH * D)
            .astype(np.float16)
        )

        # dst relative to the processed window's node base
        win_base_proc = c * NPC_REAL + ordc * W  # global node base per slot
        dstrel = np.full((E_pad,), 200.0, np.float16)
        dstrel[edest] = (
            col_s[src_idx] - np.repeat(win_base_proc, cnts)
        ).astype(np.float16)
        dstrelT = np.ascontiguousarray(dstrel.reshape(NCH, P).T)

        # node features per 64-node window slot, zero-padded per slot
        xc = np.zeros((WINDOWS, W, D), np.float16)
        for j, w in enumerate(ordc):
            n0 = c * NPC_REAL + w * W
            n1 = min(n0 + W, (c + 1) * NPC_REAL)
            xc[j, : n1 - n0] = x[n0:n1].astype(np.float16)
        xT = xc.reshape(NPC, D).T  # [D, NPC]

        # interleave per group: [edge chunks | node features]
        gsizes = _group_sizes()
        attrT = np.empty((P, NCH * D + WINDOWS * W), np.float16)
        pos = 0
        j0 = 0
        for gsz in gsizes:
            c0, c1 = int(offs[j0]), int(offs[j0 + gsz])
            wgt = (c1 - c0) * D
            attrT[:, pos : pos + wgt] = attrT_edges[:, c0 * D : c1 * D]
            pos += wgt
            attrT[:, pos : pos + gsz * W] = xT[:, j0 * W : (j0 + gsz) * W]
            pos += gsz * W
            j0 += gsz
        assert pos == attrT.shape[1] and j0 == WINDOWS

        per_core.append(
            {"attrT": np.ascontiguousarray(attrT), "dstrelT": dstrelT,
             "order": ordc}
        )
    return tuple(CB.tolist()), per_core


def _build_consts(b1, b2, b3):
    consts = np.zeros((P, 5), np.float32)
    consts[:, 0] = b1[:P]
    consts[:, 1] = b1[P:]
    consts[:, 2] = b2[:P]
    consts[:, 3] = b2[P:]
    consts[:, 4] = b3
    return consts


def _build_wts(W1, W2, W3):
    wts = np.empty((P, 4 * HIDDEN + 2 * DOUT), np.float16)
    wts[:, 0:HIDDEN] = W1[:P]
    wts[:, HIDDEN : 2 * HIDDEN] = W1[P:]
    wts[:, 2 * HIDDEN : 3 * HIDDEN] = W2[:P]
    wts[:, 3 * HIDDEN : 4 * HIDDEN] = W2[P:]
    wts[:, 4 * HIDDEN : 4 * HIDDEN + DOUT] = W3[:P]
    wts[:, 4 * HIDDEN + DOUT : 4 * HIDDEN + 2 * DOUT] = W3[P:]
    return wts


def _build_c16(CB, dstrelT):
    """fp16 consts row-block: iota ramp | dstrel."""
    CBmax = max(CB)
    NCH = int(sum(CB))
    c16 = np.empty((P, CBmax * W + NCH), np.float16)
    c16[:, 0 : CBmax * W] = np.tile(np.arange(W, dtype=np.float16), CBmax)[None, :]
    c16[:, CBmax * W :] = dstrelT
    return c16


def kernel(x, edge_index, edge_attr, W1, b1, W2, b2, W3, b3):
    CB, per_core = _host_prep(x, edge_index, edge_attr)

    key = CB
    if key not in _prog_cache:
        _prog_cache[key] = _build_program(CB)
    nc = _prog_cache[key]

    W1 = np.asarray(W1, np.float32)
    W2 = np.asarray(W2, np.float32)
    W3 = np.asarray(W3, np.float32)
    b1 = np.asarray(b1, np.float32)
    b2 = np.asarray(b2, np.float32)
    b3 = np.asarray(b3, np.float32)
    consts = _build_consts(b1, b2, b3)
    wts = _build_wts(W1, W2, W3)
    in_maps = [
        {
            "attrT": pc["attrT"],
            "c16": _build_c16(CB, pc["dstrelT"]),
            "consts": consts,
            "wts": wts,
        }
        for pc in per_core
    ]

    res = run_bass_kernel_spmd(nc, in_maps, core_ids=list(range(N_CORES)))

    out = np.empty((N_NODES, DOUT), np.float32)
    for c in range(N_CORES):
        o = res.results[c]["outT"].T.astype(np.float32).reshape(WINDOWS, W, DOUT)
        for j, w in enumerate(per_core[c]["order"]):
            n0 = c * NPC_REAL + int(w) * W
            n1 = min(n0 + W, (c + 1) * NPC_REAL)
            out[n0:n1] = o[j, : n1 - n0]
    return out


# revision 41
# speedup vs baseline: 1.3675x; 1.0709x over previous
"""NodeNet GNN message-passing kernel for 8 Trainium2 NeuronCores.

Strategy (per sharding hint): shard nodes across the 8 cores; partition
edges by destination node on the host so the scatter-mean is device-local.

Per core (12,500 real nodes, padded to 12,544 = 196 windows of 64 nodes):
  - Host sorts edges by destination, pre-scales each edge row by
    1/count(dst) (so the segment-sum directly yields the mean), pads each
    64-node window's edge list to a multiple of 128, and lays edge
    features out chunk-transposed ([128, nch*128] fp16) so all device
    DMAs are wide and contiguous.
  - Device builds, per 128-edge chunk, a [128 edge, 64 node] fp16 one-hot
    (is_equal of dst-rel against an iota ramp) and uses the TensorEngine:
    meanT[d, n] += matmul(lhsT=attr[e,d], rhs=onehot[e,n]) accumulated in
    PSUM (fp32).  Everything stays feature-major so the 3-layer MLP
    (float32r matmuls) chains without transposes:
    h1T = relu(W1.T @ [xT; meanT] + b1), etc.
  - Output is written feature-major fp16 [128, 12544]; host transposes
    and upcasts.
"""

import numpy as np

import concourse.bacc as bacc
import concourse.mybir as mybir
import concourse.tile as tile
from concourse.bass_utils import run_bass_kernel_spmd

P = 128                    # partitions / matmul contraction tile
D = 128                    # node & edge feature dim
HIDDEN = 256
DOUT = 128
N_NODES = 100000
N_CORES = 8
NPC_REAL = 12500           # real nodes per core
W = 64                     # nodes per binning window
WINDOWS = 196              # windows per core (196*64 = 12544)
NPC = WINDOWS * W          # padded nodes per core
GPW = 8                    # windows per MLP group (512 nodes)
GROUP_N = GPW * W
ATTR_BUFS = 3
OH_BUFS = 6
ACT_BUFS = 4
PBIN_BUFS = 3

_prog_cache: dict = {}

f32 = mybir.dt.float32
f16 = mybir.dt.float16
f32r = mybir.dt.float32r


def _group_sizes():
    gsizes = []
    rem = WINDOWS
    while rem > 2 * GPW:
        gsizes.append(GPW)
        rem -= GPW
    for t in (GPW // 2, GPW // 2, GPW // 4, GPW // 4):
        t = min(t, rem)
        if t > 0:
            gsizes.append(t)
            rem -= t
    while rem > 0:
        t = min(GPW // 4, rem)
        gsizes.append(t)
        rem -= t
    return gsizes


def _build_program(CB, ablate=()):
    """Build the Bass/Tile program. CB[j] = number of 128-edge chunks for
    window j (identical across cores; per-core data is padded to match).
    ablate: subset of {"mlp", "bin", "oh"} to skip stages (sim studies)."""
    CB = list(CB)
    CBmax = max(CB)
    offs = np.concatenate([[0], np.cumsum(CB)]).astype(int)
    NCH = int(offs[-1])

    nc = bacc.Bacc(None)
    # attrT carries, per group: the edge-feature chunks, then the group's
    # node features (gsz*W fp16 columns) — one combined DMA per group.
    attrT_d = nc.dram_tensor(
        "attrT", [P, NCH * D + WINDOWS * W], f16, kind="ExternalInput"
    )
    # fp16 consts: iota ramp (CBmax*W) | dstrel (NCH)
    c16_d = nc.dram_tensor("c16", [P, CBmax * W + NCH], f16, kind="ExternalInput")
    # fp32 consts: 5 bias columns
    consts_d = nc.dram_tensor("consts", [P, 5], f32, kind="ExternalInput")
    wts_d = nc.dram_tensor("wts", [P, 4 * HIDDEN + 2 * DOUT], f16,
                           kind="ExternalInput")
    outT_d = nc.dram_tensor("outT", [P, NPC], f16, kind="ExternalOutput")

    Relu = mybir.ActivationFunctionType.Relu
    Ident = mybir.ActivationFunctionType.Identity

    with tile.TileContext(nc) as tc:
        with (
            tc.tile_pool(name="const", bufs=1) as cpool,
            tc.tile_pool(name="attr", bufs=ATTR_BUFS) as apool,
            tc.tile_pool(name="oh", bufs=OH_BUFS) as ohpool,
            tc.tile_pool(name="acts", bufs=ACT_BUFS) as actpool,
            tc.tile_pool(name="pbin", bufs=PBIN_BUFS, space="PSUM") as pbin,
            tc.tile_pool(name="pmlp", bufs=1, space="PSUM") as pmlp,
        ):
            # --- constants (tiles now; DMAs after the first attr DMA so
            # the edge stream starts immediately) ---
            cs = cpool.tile([P, 5], f32, tag="consts")
            ws = cpool.tile([P, 4 * HIDDEN + 2 * DOUT], f16, tag="wts")
            c16 = cpool.tile([P, CBmax * W + NCH], f16, tag="c16")
            w1s_0 = ws[:, 0:HIDDEN]
            w1s_1 = ws[:, HIDDEN : 2 * HIDDEN]
            w2s_0 = ws[:, 2 * HIDDEN : 3 * HIDDEN]
            w2s_1 = ws[:, 3 * HIDDEN : 4 * HIDDEN]
            w3s_0 = ws[:, 4 * HIDDEN : 4 * HIDDEN + DOUT]
            w3s_1 = ws[:, 4 * HIDDEN + DOUT : 4 * HIDDEN + 2 * DOUT]
            b1s_0 = cs[:, 0:1]
            b1s_1 = cs[:, 1:2]
            b2s_0 = cs[:, 2:3]
            b2s_1 = cs[:, 3:4]
            b3s = cs[:, 4:5]
            it16 = c16[:, 0 : CBmax * W]
            dstrel_s = c16[:, CBmax * W : CBmax * W + NCH]
            oall = cpool.tile([P, NPC], f16, tag="oall")

            # group sizes: GPW windows each, tapering at the tail to
            # shorten the pipeline drain (last windows are also the
            # smallest thanks to the descending-count permutation)
            gsizes = _group_sizes()
            gstart = [0]
            for s in gsizes:
                gstart.append(gstart[-1] + s)

            for j in range(WINDOWS):
                cb = CB[j]
                off = int(offs[j])
                g = next(i for i in range(len(gsizes)) if gstart[i + 1] > j)
                sw = j - gstart[g]
                gsz = gsizes[g]

                if sw == 0:
                    # one combined edge-feature + node-feature DMA per group
                    goff = off
                    gend = int(offs[gstart[g + 1]])
                    gw = (gend - goff) * D + gsz * W
                    gsrc = goff * D + gstart[g] * W
                    at = apool.tile([P, GPW * (CBmax * D + W)], f16, tag="attr")
                    nc.sync.dma_start(
                        out=at[:, :gw], in_=attrT_d[:, gsrc : gsrc + gw]
                    )
                    if j == 0:
                        nc.sync.dma_start(out=c16[:], in_=c16_d[:, :])
                        nc.sync.dma_start(out=cs[:], in_=consts_d[:, :])
                        nc.sync.dma_start(out=ws[:], in_=wts_d[:, :])
                    # flush the previous group's finished output slice
                    if g > 0 and gsizes[g - 1] == GPW:
                        f0, f1 = gstart[g - 1] * W, gstart[g] * W
                        nc.sync.dma_start(
                            out=outT_d[:, f0:f1], in_=oall[:, f0:f1]
                        )
                woff = off - goff  # window's chunk offset within group tile

                oh = ohpool.tile([P, CBmax * W], f16, tag="oh")
                if "oh" not in ablate:
                    nc.vector.tensor_tensor(
                        out=oh[:, : cb * W].rearrange("p (c m) -> p c m", m=W),
                        in0=dstrel_s[:, off : off + cb].to_broadcast([P, cb, W]),
                        in1=it16[:, : cb * W].rearrange("p (c m) -> p c m", m=W),
                        op=mybir.AluOpType.is_equal,
                    )

                pm = pbin.tile([P, W], f32, tag="mean")
                for ch in range(cb if "bin" not in ablate else 0):
                    nc.tensor.matmul(
                        out=pm[:],
                        lhsT=at[:, (woff + ch) * D : (woff + ch + 1) * D],
                        rhs=oh[:, ch * W : (ch + 1) * W],
                        start=(ch == 0),
                        stop=(ch == cb - 1),
                    )

                if sw == 0:
                    mean_g = actpool.tile([P, GROUP_N], f16, tag="mean_g")
                if "bin" not in ablate:
                    nc.scalar.copy(out=mean_g[:, sw * W : (sw + 1) * W], in_=pm[:])

                if ("mlp" not in ablate) and (sw == gsz - 1):
                    # --- MLP over this group of nodes (feature-major) ---
                    NW = gsz * W
                    n0 = gstart[g] * W

                    ph1a = pmlp.tile([P, GROUP_N], f32, tag="h1a")
                    ph1b = pmlp.tile([P, GROUP_N], f32, tag="h1b")
                    nc.tensor.matmul(out=ph1a[:, :NW], lhsT=w1s_0[:, 0:P],
                                     rhs=at[:, (gend - goff) * D : (gend - goff) * D + NW], start=True, stop=False)
                    nc.tensor.matmul(out=ph1a[:, :NW], lhsT=w1s_1[:, 0:P],
                                     rhs=mean_g[:, :NW], start=False, stop=True)
                    nc.tensor.matmul(out=ph1b[:, :NW], lhsT=w1s_0[:, P:HIDDEN],
                                     rhs=at[:, (gend - goff) * D : (gend - goff) * D + NW], start=True, stop=False)
                    nc.tensor.matmul(out=ph1b[:, :NW], lhsT=w1s_1[:, P:HIDDEN],
                                     rhs=mean_g[:, :NW], start=False, stop=True)
                    h1a = actpool.tile([P, GROUP_N], f16, tag="h1a_s")
                    h1b = actpool.tile([P, GROUP_N], f16, tag="h1b_s")
                    nc.scalar.activation(out=h1a[:, :NW], in_=ph1a[:, :NW],
                                         func=Relu, bias=b1s_0[:, 0:1])
                    nc.scalar.activation(out=h1b[:, :NW], in_=ph1b[:, :NW],
                                         func=Relu, bias=b1s_1[:, 0:1])

                    ph2a = pmlp.tile([P, GROUP_N], f32, tag="h2a")
                    ph2b = pmlp.tile([P, GROUP_N], f32, tag="h2b")
                    nc.tensor.matmul(out=ph2a[:, :NW], lhsT=w2s_0[:, 0:P],
                                     rhs=h1a[:, :NW], start=True, stop=False)
                    nc.tensor.matmul(out=ph2a[:, :NW], lhsT=w2s_1[:, 0:P],
                                     rhs=h1b[:, :NW], start=False, stop=True)
                    nc.tensor.matmul(out=ph2b[:, :NW], lhsT=w2s_0[:, P:HIDDEN],
                                     rhs=h1a[:, :NW], start=True, stop=False)
                    nc.tensor.matmul(out=ph2b[:, :NW], lhsT=w2s_1[:, P:HIDDEN],
                                     rhs=h1b[:, :NW], start=False, stop=True)
                    h2a = actpool.tile([P, GROUP_N], f16, tag="h2a_s")
                    h2b = actpool.tile([P, GROUP_N], f16, tag="h2b_s")
                    nc.scalar.activation(out=h2a[:, :NW], in_=ph2a[:, :NW],
                                         func=Relu, bias=b2s_0[:, 0:1])
                    nc.scalar.activation(out=h2b[:, :NW], in_=ph2b[:, :NW],
                                         func=Relu, bias=b2s_1[:, 0:1])

                    po = pmlp.tile([P, GROUP_N], f32, tag="o")
                    nc.tensor.matmul(out=po[:, :NW], lhsT=w3s_0[:],
                                     rhs=h2a[:, :NW], start=True, stop=False)
                    nc.tensor.matmul(out=po[:, :NW], lhsT=w3s_1[:],
                                     rhs=h2b[:, :NW], start=False, stop=True)
                    nc.scalar.activation(out=oall[:, n0 : n0 + NW],
                                         in_=po[:, :NW],
                                         func=Ident, bias=b3s[:, 0:1])
                    if gsz < GPW:
                        # tail taper groups: no more prefetches to protect,
                        # store immediately to shorten the drain
                        nc.sync.dma_start(
                            out=outT_d[:, n0 : n0 + NW], in_=oall[:, n0 : n0 + NW]
                        )

            if gsizes[-1] == GPW:
                f0 = gstart[len(gsizes) - 1] * W
                nc.sync.dma_start(out=outT_d[:, f0:], in_=oall[:, f0:])

    # run_bass_via_pjrt (axon path) does not finalize; Bacc needs
    # finalize() to run its compile passes (reg alloc, wait legalization).
    nc.finalize()
    return nc


def _host_prep(x, edge_index, edge_attr):
    """Sort/scale/pad edges; returns (CB, per-core input arrays)."""
    col = np.asarray(edge_index)[1].astype(np.int64)
    x = np.asarray(x, dtype=np.float32)
    counts = np.bincount(col, minlength=N_NODES)
    scale = (1.0 / np.maximum(counts, 1)).astype(np.float32)

    order = np.argsort(col, kind="stable")
    col_s = col[order]
    attr_s = np.asarray(edge_attr, dtype=np.float32)[order]
    attr_s = attr_s * scale[col_s][:, None]

    # per-core, per-window edge counts
    starts = np.empty((N_CORES, WINDOWS + 1), dtype=np.int64)
    for c in range(N_CORES):
        bounds = np.minimum(
            c * NPC_REAL + np.arange(WINDOWS + 1) * W, (c + 1) * NPC_REAL
        )
        starts[c] = np.searchsorted(col_s, bounds)
    cnt = np.diff(starts, axis=1)  # [N_CORES, WINDOWS]

    # Each core processes its windows sorted by descending edge count.
    # Window slot j then holds every core's j-th order statistic, so the
    # cross-core max (CB must be shared, the program is SPMD) wastes far
    # less padding than positional assignment.  Small windows land last,
    # which also shortens the pipeline drain.  Host un-permutes outputs.
    order = np.argsort(-cnt, axis=1, kind="stable")  # [N_CORES, WINDOWS]
    cnt_s = np.take_along_axis(cnt, order, axis=1)

    CB = np.maximum(1, (-(-cnt_s // P)).max(axis=0)).astype(int)  # ceil, >=1
    offs = np.concatenate([[0], np.cumsum(CB)]).astype(np.int64)
    NCH = int(offs[-1])
    E_pad = NCH * P

    per_core = []
    for c in range(N_CORES):
        ordc = order[c]
        cnts = cnt_s[c]                      # counts in processing order
        total = int(cnts.sum())
        # edge source rows (into col_s/attr_s), in processing order
        src_idx = np.concatenate(
            [np.arange(starts[c, w], starts[c, w + 1]) for w in ordc]
        )
        base = np.repeat(offs[:-1] * P, cnts)
        within = np.arange(total) - np.repeat(np.cumsum(cnts) - cnts, cnts)
        edest = base + within

        attr_pad = np.zeros((E_pad, D), np.float32)
        attr_pad[edest] = attr_s[src_idx]
        attrT_edges = (
            attr_pad.reshape(NCH, P, D)
            .transpose(1, 0, 2)
            .reshape(P, NCH * D)
            .astype(np.float16)
        )

        # dst relative to the processed window's node base
        win_base_proc = c * NPC_REAL + ordc * W  # global node base per slot
        dstrel = np.full((E_pad,), 200.0, np.float16)
        dstrel[edest] = (
            col_s[src_idx] - np.repeat(win_base_proc, cnts)
        ).astype(np.float16)
        dstrelT = np.ascontiguousarray(dstrel.reshape(NCH, P).T)

        # node features per 64-node window slot, zero-padded per slot
        xc = np.zeros((WINDOWS, W, D), np.float16)
        for j, w in enumerate(ordc):
            n0 = c * NPC_REAL + w * W
            n1 = min(n0 + W, (c + 1) * NPC_REAL)
            xc[j, : n1 - n0] = x[n0:n1].astype(np.float16)
        xT = xc.reshape(NPC, D).T  # [D, NPC]

        # interleave per group: [edge chunks | node features]
        gsizes = _group_sizes()
        attrT = np.empty((P, NCH * D + WINDOWS * W), np.float16)
        pos = 0
        j0 = 0
        for gsz in gsizes:
            c0, c1 = int(offs[j0]), int(offs[j0 + gsz])
            wgt = (c1 - c0) * D
            attrT[:, pos : pos + wgt] = attrT_edges[:, c0 * D : c1 * D]
            pos += wgt
            attrT[:, pos : pos + gsz * W] = xT[:, j0 * W : (j0 + gsz) * W]
            pos += gsz * W
            j0 += gsz
        assert pos == attrT.shape[1] and j0 == WINDOWS

        per_core.append(
            {"attrT": np.ascontiguousarray(attrT), "dstrelT": dstrelT,
             "order": ordc}
        )
    return tuple(CB.tolist()), per_core


def _build_consts(b1, b2, b3):
    consts = np.zeros((P, 5), np.float32)
    consts[:, 0] = b1[:P]
    consts[:, 1] = b1[P:]
    consts[:, 2] = b2[:P]
    consts[:, 3] = b2[P:]
    consts[:, 4] = b3
    return consts


def _build_wts(W1, W2, W3):
    wts = np.empty((P, 4 * HIDDEN + 2 * DOUT), np.float16)
    wts[:, 0:HIDDEN] = W1[:P]
    wts[:, HIDDEN : 2 * HIDDEN] = W1[P:]
    wts[:, 2 * HIDDEN : 3 * HIDDEN] = W2[:P]
    wts[:, 3 * HIDDEN : 4 * HIDDEN] = W2[P:]
    wts[:, 4 * HIDDEN : 4 * HIDDEN + DOUT] = W3[:P]
    wts[:, 4 * HIDDEN + DOUT : 4 * HIDDEN + 2 * DOUT] = W3[P:]
    return wts


def _build_c16(CB, dstrelT):
    """fp16 consts row-block: iota ramp | dstrel."""
    CBmax = max(CB)
    NCH = int(sum(CB))
    c16 = np.empty((P, CBmax * W + NCH), np.float16)
    c16[:, 0 : CBmax * W] = np.tile(np.arange(W, dtype=np.float16), CBmax)[None, :]
    c16[:, CBmax * W :] = dstrelT
    return c16


def kernel(x, edge_index, edge_attr, W1, b1, W2, b2, W3, b3):
    CB, per_core = _host_prep(x, edge_index, edge_attr)

    key = CB
    if key not in _prog_cache:
        _prog_cache[key] = _build_program(CB)
    nc = _prog_cache[key]

    W1 = np.asarray(W1, np.float32)
    W2 = np.asarray(W2, np.float32)
    W3 = np.asarray(W3, np.float32)
    b1 = np.asarray(b1, np.float32)
    b2 = np.asarray(b2, np.float32)
    b3 = np.asarray(b3, np.float32)
    consts = _build_consts(b1, b2, b3)
    wts = _build_wts(W1, W2, W3)
    in_maps = [
        {
            "attrT": pc["attrT"],
            "c16": _build_c16(CB, pc["dstrelT"]),
            "consts": consts,
            "wts": wts,
        }
        for pc in per_core
    ]

    res = run_bass_kernel_spmd(nc, in_maps, core_ids=list(range(N_CORES)))

    out = np.empty((N_NODES, DOUT), np.float32)
    for c in range(N_CORES):
        o = res.results[c]["outT"].T.astype(np.float32).reshape(WINDOWS, W, DOUT)
        for j, w in enumerate(per_core[c]["order"]):
            n0 = c * NPC_REAL + int(w) * W
            n1 = min(n0 + W, (c + 1) * NPC_REAL)
            out[n0:n1] = o[j, : n1 - n0]
    return out
